# revision 30
# baseline (speedup 1.0000x reference)
"""AttentionBlock (GroupNorm + single-head full attention + residual) on 8 trn2 cores.

Sharding: core i -> batch i//4, query strip (i%4)*1024 .. +1024. Each core
computes its batch's full K/V (duplicated across the 4 cores sharing the
batch) so no inter-core communication is needed. The host rotates each
core's copy of x so its query strip sits at token rows 0..1023 (group-norm
statistics and attention key-sums are permutation-invariant over tokens),
which lets one SPMD program serve all cores.

Changes vs the bf16 baseline (292-352us) -> ~195us measured:
  - x arrives channel-major (host-side transpose): no PE transposes and no
    ones/Square stats matmuls. Group-norm stats: DVE bn_stats/bn_aggr for
    chunks 0-2; chunk 3 entirely on ScalarE via two Square+accum_out
    passes (sum(x) recovered from sum((x+1)^2)-sum(x^2)-n). Group combine
    and per-channel redistribution are tiny 8/128-partition matmuls
    (1/16 folded into the host sel8 matrix); rstd is plain sqrt+reciprocal
    (~4e-3, inside the fp8 error budget).
  - The Q projection is folded into the K side on the host (when bq == 0):
    Z = h @ (wk wq^T * C^-0.5), so scores S^T = Z^T . h_q use raw
    normalized h on the query side. One less projection pass.
  - The attention core runs in fp8 e4m3 with DoubleRow double-pumped
    matmuls (2 contraction chunks per instruction, issue rate measured at
    the 216ns N=512 streaming floor = 2x bf16 math): scores, exp row-sums
    and P^T.V. Z^T / V / h_q are quantized to e4m3 at PSUM evacuation.
    exp(s - 5) keeps P in e4m3 range (logits measured in [-7.5, 7.2];
    e4m3 covers [2e-3, 240]). Softmax normalization is deferred to the
    f32r projection output, so fp8 rowsum noise largely cancels.
  - P3 is software-pipelined per 256-key pair: scores/exp for pair p while
    the rowsum + P^T.V DoubleRow matmuls consume pair p-1, so the PE never
    paces on the ScalarE exp drain. PSUM: 4 ot banks + 1 rowsum + 3 score.
  - Projections (Z, V) stay bf16 (fp8 projections fail the 2e-2 gate:
    measured 5e-2); out-projection stays float32r. Evacuations are spread
    DVE/ScalarE/GpSimd; P3 ScalarE stays pure-Exp (no act-table reloads).
End-to-end absmax-relative error vs the fp32 reference: 6.1e-3 on HW
(tolerance 2e-2). HAM keep-alive matmuls hold the PE clock at 2.4GHz.
"""

import numpy as np
from contextlib import ExitStack

import concourse.bass as bass
import concourse.bacc as bacc
import concourse.tile as tile
from concourse import mybir
from concourse.bass_utils import run_bass_kernel_spmd

B, H, W, C = 2, 64, 64, 512
T = H * W                 # 4096 tokens per batch
NCORES = 8
QS = 1024                 # queries per core
GROUPS, GSIZE = 32, 16    # 8 groups per 128-channel chunk
EPS = 1e-5
SCALE = float(C) ** -0.5
SHIFT = 5.0               # softmax logit shift so exp() fits e4m3
F32 = mybir.dt.float32
F32R = mybir.dt.float32r
BF16 = mybir.dt.bfloat16
E4 = mybir.dt.float8e4
DR = mybir.MatmulPerfMode.DoubleRow
NCH = C // 128            # 4 channel chunks
NW = T // 512             # 8 token windows per batch
NBLK = QS // 512          # 2 attention q-blocks of 512 queries
NSUB = 4                  # 128-query subtiles per block
NKP = T // 256            # 16 key-tile pairs per q-block


def _build(fold_q: bool):
    nc = bacc.Bacc(None, target_bir_lowering=False)

    xt_h = nc.declare_dram_parameter("xt", [C, T], BF16, isOutput=False)
    xres_h = nc.declare_dram_parameter("xres", [QS, C], F32, isOutput=False)
    g_h = nc.declare_dram_parameter("gmat", [C, C], BF16, isOutput=False)
    wv8_h = nc.declare_dram_parameter("wv8", [2, 128, 2, C], E4, isOutput=False)
    wp8_h = nc.declare_dram_parameter("wp8", [2, 128, 2, C], E4, isOutput=False)
    bpp_h = nc.declare_dram_parameter("bpp", [C], F32, isOutput=False)
    gamma_h = nc.declare_dram_parameter("gamma", [C], F32, isOutput=False)
    beta_h = nc.declare_dram_parameter("beta", [C], F32, isOutput=False)
    sel8_h = nc.declare_dram_parameter("sel8", [128, 8], F32, isOutput=False)
    repl8_h = nc.declare_dram_parameter("repl8", [8, 128], F32, isOutput=False)
    if fold_q:
        wq_h = nc.declare_dram_parameter("wq", [C, C], BF16, isOutput=False)
        bq_h = nc.declare_dram_parameter("bq", [C], F32, isOutput=False)
    out_h = nc.declare_dram_parameter("out", [QS, C], F32, isOutput=True)

    with tile.TileContext(nc) as tc, ExitStack() as ctx:
        persist = ctx.enter_context(tc.tile_pool(name="persist", bufs=1))
        small = ctx.enter_context(tc.tile_pool(name="small", bufs=1))

        bigpool = ctx.enter_context(tc.tile_pool(name="bigpool", bufs=1))
        xt_t = [bigpool.tile([128, T], BF16, tag=f"xt{j}", name=f"xt{j}") for j in range(NCH)]
        # fp8 operand tiles in DoubleRow pair layout [128, 2, ...]
        zt2 = [bigpool.tile([128, 2, T], E4, tag=f"zt{c}", name=f"zt{c}") for c in range(2)]
        h8 = [bigpool.tile([128, 2, T], E4, tag=f"h8{c}", name=f"h8{c}") for c in range(2)]
        v_big = bigpool.tile([128, T // 128, C], E4, tag="vbig", name="vbig")

        ctx2 = ExitStack()
        wpool = ctx2.enter_context(tc.tile_pool(name="wpool", bufs=1))
        g_t = [wpool.tile([128, C], BF16, tag=f"g{j}", name=f"g{j}") for j in range(NCH)]
        wv8_t = [wpool.tile([128, 2, C], E4, tag=f"wv8{j}", name=f"wv8{j}") for j in range(2)]
        wp8_t = [persist.tile([128, 2, C], E4, tag=f"wp8{j}", name=f"wp8{j}") for j in range(2)]
        if fold_q:
            wq_t = [wpool.tile([128, C], BF16, tag=f"wq{j}", name=f"wq{j}") for j in range(NCH)]
        for j in range(NCH):
            sl = slice(j * 128, (j + 1) * 128)
            nc.scalar.dma_start(out=g_t[j], in_=g_h[sl, :])
            if fold_q:
                nc.scalar.dma_start(out=wq_t[j], in_=wq_h[sl, :])
        for cp in range(2):
            nc.scalar.dma_start(out=wv8_t[cp], in_=wv8_h[cp, :, :, :])
            nc.scalar.dma_start(out=wp8_t[cp], in_=wp8_h[cp, :, :, :])

        # per-channel vectors as [128, NCH] (column j = channel chunk j)
        def vec_tile(h, name):
            t = small.tile([128, NCH], F32, tag=name)
            nc.scalar.dma_start(out=t, in_=h.rearrange("(a p) -> p a", p=128))
            return t

        gamma_sb = vec_tile(gamma_h, "gamma")
        beta_sb = vec_tile(beta_h, "beta")
        bpp_row = small.tile([1, C], F32, tag="bpprow", name="bpprow")
        nc.scalar.dma_start(out=bpp_row, in_=bpp_h.rearrange("(a c) -> a c", a=1))
        sel8 = small.tile([128, 8], F32, tag="sel8", name="sel8")
        nc.sync.dma_start(out=sel8, in_=sel8_h[:, :])
        repl8 = small.tile([8, 128], F32, tag="repl8", name="repl8")
        nc.sync.dma_start(out=repl8, in_=repl8_h[:, :])
        if fold_q:
            qts2 = [bigpool.tile([128, 2, QS], E4, tag=f"qts{c}", name=f"qts{c}") for c in range(2)]
            bq_sb = vec_tile(bq_h, "bq")
            sbq = small.tile([128, NCH], F32, tag="sbq", name="sbq")
            nc.vector.tensor_scalar_mul(sbq, bq_sb, SCALE)

        ones1 = small.tile([1, 1], F32, tag="ones1", name="ones1")
        nc.vector.memset(ones1, 1.0)
        nshift = small.tile([128, 1], F32, tag="nshift", name="nshift")
        nc.vector.memset(nshift, -SHIFT)
        onesd = small.tile([128, 2, 16], E4, tag="onesd", name="onesd")
        nc.vector.memset(onesd, 1.0)

        rinv_t = [small.tile([128, 1], F32, tag=f"rinv{s}", name=f"rinv{s}") for s in range(NSUB * NBLK)]
        scale_t = [small.tile([128, 1], F32, tag=f"gnsc{j}", name=f"gnsc{j}") for j in range(NCH)]
        bias_t = [small.tile([128, 1], F32, tag=f"gnbi{j}", name=f"gnbi{j}") for j in range(NCH)]

        # PE warm-up / keep-alive dummy matmuls (HAM unthrottle 1.2->2.4GHz)
        warm_sb = small.tile([128, 512], BF16, tag="warm_sb", name="warm_sb")
        nc.vector.memset(warm_sb, 0.0)

        ctxkeep = ExitStack()
        p1ps_keep = ctxkeep.enter_context(tc.tile_pool(name="keepps", bufs=1, space="PSUM"))

        def keepalive(n, lhs=None):
            for _ in range(n):
                kps = p1ps_keep.tile([128, 512], F32, tag="keep", name="keep", bufs=1)
                if lhs is None:
                    nc.tensor.matmul(kps, warm_sb[:, 0:128], warm_sb,
                                     start=True, stop=True)
                else:
                    nc.tensor.matmul(kps[0:1, :], lhs, warm_sb,
                                     start=True, stop=True)

        # ================= P1: stream XT, bn_stats group statistics =========
        # Per-chunk pipeline (a chunk's group scale/bias only depends on its
        # own 128 channels): half-chunk DMAs -> 3D bn_stats -> bn_aggr ->
        # tiny 8/128-partition matmuls for the 16-channel group combine and
        # per-channel redistribution.
        with tc.tile_pool(name="p1ps", bufs=1, space="PSUM") as p1ps, \
             tc.tile_pool(name="p1sb", bufs=1) as p1sb:
            keepalive(18)
            HT = T // 2
            # DMA halves interleaved so the ScalarE/reduce chunk (3) lands
            # early while the DVE bn_stats chunks stream in order
            for hf in range(2):
                for j in (3, 0, 1, 2):
                    sl = slice(hf * HT, (hf + 1) * HT)
                    nc.sync.dma_start(out=xt_t[j][:, sl],
                                      in_=xt_h[j * 128:(j + 1) * 128, sl])

            Sjs = []
            # chunk 3 entirely on ScalarE: two Square+accum passes per slice
            # (bias 0 and bias 1; one act-table segment). Algebra recovers the
            # plain sums: sum(x) = (sum((x+1)^2) - sum(x^2) - 512) / 2.
            onescol = small.tile([128, 1], F32, tag="onescol", name="onescol")
            nc.vector.memset(onescol, 1.0)
            ssq8 = p1sb.tile([128, NW], F32, tag="ssq8", name="ssq8")
            sqb8 = p1sb.tile([128, NW], F32, tag="sqb8", name="sqb8")
            for s in range(NW):
                scr = p1sb.tile([128, 512], BF16, tag="sqscr", name="sqscr", bufs=2)
                nc.scalar.activation(scr, xt_t[3][:, s * 512:(s + 1) * 512],
                                     mybir.ActivationFunctionType.Square,
                                     accum_out=ssq8[:, s:s + 1])
                scr2 = p1sb.tile([128, 512], BF16, tag="sqscr", name="sqscr", bufs=2)
                nc.scalar.activation(scr2, xt_t[3][:, s * 512:(s + 1) * 512],
                                     mybir.ActivationFunctionType.Square,
                                     bias=onescol,
                                     accum_out=sqb8[:, s:s + 1])
            # chunks 0-2: DVE bn_stats / bn_aggr
            for j in range(3):
                bns = p1sb.tile([128, NW, 6], F32, tag=f"bns{j}", name=f"bns{j}")
                for s in range(NW):
                    nc.vector.bn_stats(bns[:, s, :],
                                       xt_t[j][:, s * 512:(s + 1) * 512])
                mvj = p1sb.tile([128, 2], F32, tag=f"mv{j}", name=f"mv{j}")
                nc.vector.bn_aggr(mvj, bns)
                # Sj: col 0 = mean_c, col 1 = E[x^2]_c
                Sj = p1sb.tile([128, 2], F32, tag=f"S{j}", name=f"S{j}")
                nc.vector.tensor_copy(Sj[:, 0:1], mvj[:, 0:1])
                nc.vector.scalar_tensor_tensor(
                    out=Sj[:, 1:2], in0=mvj[:, 0:1], scalar=mvj[:, 0:1],
                    in1=mvj[:, 1:2], op0=mybir.AluOpType.mult,
                    op1=mybir.AluOpType.add)
                Sjs.append(Sj)
                if j in (0, 1, 2):
                    wj = p1sb.tile([128, 1], BF16, tag=f"warm{j}", name=f"warm{j}")
                    nc.gpsimd.tensor_copy(wj, mvj[:, 0:1])
                    keepalive(3, lhs=wj)
            S3 = p1sb.tile([128, 2], F32, tag="S3", name="S3")
            dsq = p1sb.tile([128, NW], F32, tag="dsq", name="dsq")
            nc.vector.tensor_tensor(out=dsq, in0=sqb8, in1=ssq8,
                                    op=mybir.AluOpType.subtract)
            nc.vector.tensor_reduce(out=S3[:, 0:1], in_=dsq,
                                    axis=mybir.AxisListType.X, op=mybir.AluOpType.add)
            # sum(dsq) = 2*sum(x) + T  ->  mean = (sum(dsq) - T) / (2T)
            nc.vector.tensor_scalar(out=S3[:, 0:1], in0=S3[:, 0:1],
                                    scalar1=-float(T), scalar2=0.5 / T,
                                    op0=mybir.AluOpType.add,
                                    op1=mybir.AluOpType.mult)
            nc.vector.tensor_reduce(out=S3[:, 1:2], in_=ssq8,
                                    axis=mybir.AxisListType.X, op=mybir.AluOpType.add)
            nc.vector.tensor_scalar_mul(S3[:, 1:2], S3[:, 1:2], 1.0 / T)
            Sjs.append(S3)
            w3 = p1sb.tile([128, 1], BF16, tag="warm3", name="warm3")
            nc.gpsimd.tensor_copy(w3, S3[:, 0:1])
            keepalive(3, lhs=w3)
            # pre-load the Exp activation table so the first P3 exp doesn't
            # pay the ~1.3us table switch inside the score pipeline
            expwarm = p1sb.tile([128, 1], F32, tag="expwarm", name="expwarm")
            nc.scalar.activation(expwarm, nshift,
                                 mybir.ActivationFunctionType.Exp)

            # per chunk: group-combine matmul (1/16 folded into sel8),
            # plain sqrt+reciprocal rstd (~4e-3, inside the fp8 budget),
            # redistribution matmul, then per-channel scale/bias
            for j, Sj in zip((0, 1, 2, 3), Sjs):
                g8_ps = p1ps.tile([8, 2], F32, tag="g8", name="g8", bufs=2)
                nc.tensor.matmul(g8_ps, sel8, Sj, start=True, stop=True)
                vals = p1sb.tile([8, 2], F32, tag=f"vals{j}", name=f"vals{j}")
                nc.vector.tensor_copy(vals, g8_ps)
                msq8 = p1sb.tile([8, 1], F32, tag="msq8", name="msq8")
                nc.vector.tensor_tensor(out=msq8, in0=vals[:, 0:1], in1=vals[:, 0:1],
                                        op=mybir.AluOpType.mult)
                ve = p1sb.tile([8, 1], F32, tag="ve", name="ve")
                nc.vector.tensor_tensor(out=ve, in0=vals[:, 1:2], in1=msq8,
                                        op=mybir.AluOpType.subtract)
                nc.vector.tensor_scalar_add(ve, ve, EPS)
                sd = p1sb.tile([8, 1], F32, tag="sd", name="sd")
                nc.scalar.activation(sd, ve, mybir.ActivationFunctionType.Sqrt)
                nc.vector.reciprocal(vals[:, 1:2], sd)
                b128_ps = p1ps.tile([128, 2], F32, tag="b128", name="b128", bufs=2)
                nc.tensor.matmul(b128_ps, repl8, vals, start=True, stop=True)
                bc = p1sb.tile([128, 2], F32, tag=f"bc{j}", name=f"bc{j}")
                nc.vector.tensor_copy(bc, b128_ps)
                nc.vector.tensor_tensor(out=scale_t[j], in0=bc[:, 1:2],
                                        in1=gamma_sb[:, j:j + 1],
                                        op=mybir.AluOpType.mult)
                mt = p1sb.tile([128, 1], F32, tag="mt", name="mt")
                nc.vector.tensor_tensor(out=mt, in0=bc[:, 0:1], in1=scale_t[j],
                                        op=mybir.AluOpType.mult)
                nc.vector.tensor_tensor(out=bias_t[j], in0=beta_sb[:, j:j + 1],
                                        in1=mt, op=mybir.AluOpType.subtract)

        keepalive(4)
        ctxkeep.close()

        # ================= P2: normalize window-pairs -> Z^T, V (+ Q^T) =====
        # 1024-token moving operands (bf16 moving max) into 2-bank PSUM
        # tiles: half the matmul/LDWEIGHTS/evac instruction count.
        with tc.tile_pool(name="p2ps", bufs=1, space="PSUM") as p2ps, \
             tc.tile_pool(name="p2sb", bufs=2) as p2sb:
            for wp in range(NW // 2):
                t0 = wp * 1024
                # normalize straight into the e4m3 pair-layout h8 (no bf16
                # h at all: Z runs bf16-weights x e4m3-h, V/scores run fp8)
                for j in range(3):
                    nc.vector.tensor_scalar(
                        out=h8[j // 2][:, j % 2, t0:t0 + 1024],
                        in0=xt_t[j][:, t0:t0 + 1024],
                        scalar1=scale_t[j], scalar2=bias_t[j],
                        op0=mybir.AluOpType.mult, op1=mybir.AluOpType.add)
                nc.scalar.activation(
                    h8[1][:, 1, t0:t0 + 1024], xt_t[3][:, t0:t0 + 1024],
                    mybir.ActivationFunctionType.Identity,
                    bias=bias_t[3], scale=scale_t[3])
                for ck in range(NCH):
                    ps2 = p2ps.tile([128, 2, 512], F32, tag="zp", name="zp", bufs=2)
                    for hh in range(2):
                        hs = t0 + hh * 512
                        for ci in range(NCH):
                            nc.tensor.matmul(
                                ps2[:, hh, :], g_t[ci][:, ck * 128:(ck + 1) * 128],
                                h8[ci // 2][:, ci % 2, hs:hs + 512],
                                start=(ci == 0), stop=(ci == NCH - 1))
                    nc.vector.tensor_copy(zt2[ck // 2][:, ck % 2, t0:t0 + 1024], ps2)
                for m in range(4):
                    ps2 = p2ps.tile([128, 2, 512], F32, tag="vp", name="vp", bufs=2)
                    for hh in range(2):
                        tb = wp * 8 + 2 * m + hh
                        for cp in range(2):
                            nc.tensor.matmul(
                                ps2[:, hh, :],
                                h8[cp][:, :, tb * 128:(tb + 1) * 128],
                                wv8_t[cp], start=(cp == 0), stop=(cp == 1),
                                perf_mode=DR)
                    nc.scalar.activation(
                        v_big[:, wp * 8 + 2 * m:wp * 8 + 2 * m + 2, :], ps2,
                        mybir.ActivationFunctionType.Identity)
                if wp == 0 and fold_q:
                    for cq in range(NCH):
                        ps2 = p2ps.tile([128, 2, 512], F32, tag="zp", name="zp", bufs=2)
                        for hh in range(2):
                            hs = t0 + hh * 512
                            for ci in range(NCH):
                                nc.tensor.matmul(
                                    ps2[:, hh, :], wq_t[ci][:, cq * 128:(cq + 1) * 128],
                                    h8[ci // 2][:, ci % 2, hs:hs + 512],
                                    start=(ci == 0), stop=(ci == NCH - 1))
                        nc.scalar.activation(
                            qts2[cq // 2][:, cq % 2, :], ps2,
                            mybir.ActivationFunctionType.Identity,
                            bias=sbq[:, cq:cq + 1], scale=SCALE)
        ctx2.close()

        # ================= P3: fp8 DoubleRow attention =======================
        otspool = ctx.enter_context(tc.tile_pool(name="otspool", bufs=1))
        ots8_t = [[otspool.tile([128, 2, 512], E4, tag=f"ots{b}_{cp}", name=f"ots{b}_{cp}")
                   for cp in range(2)] for b in range(NBLK)]
        with tc.tile_pool(name="p3ps", bufs=1, space="PSUM") as p3ps, \
             tc.tile_pool(name="p3ot", bufs=1, space="PSUM") as p3ot, \
             tc.tile_pool(name="p3sb", bufs=1) as p3sb, \
             tc.tile_pool(name="p3ac", bufs=4) as p3ac:
            # bias vector bp' = bv @ wp + bp (host-folded), broadcast
            bppb = p3sb.tile([128, C], F32, tag="bppb", name="bppb")
            nc.gpsimd.partition_broadcast(bppb, bpp_row[0:1, :])

            for blk in range(NBLK):
                q0 = blk * 512
                ot_ps = p3ot.tile([128, NCH, 512], F32, tag="ot", name="ot", bufs=1)
                rs_ps = p3ot.tile([1, 512], F32, tag="rsum", name="rsum", bufs=1)
                pts = [None] * NKP

                # software-pipelined: scores/exp for key-pair p while the
                # rowsum + P^T.V matmuls consume pair p-1 (PE stays busy
                # instead of pacing on the ScalarE exp drain)
                def scores_pair(p):
                    for hh in range(2):
                        w2 = 2 * p + hh
                        st_ps = p3ps.tile([128, 512], F32, tag="sc", name="st_ps", bufs=3)
                        qsrc = qts2 if fold_q else h8
                        for c2 in range(2):
                            nc.tensor.matmul(
                                st_ps, zt2[c2][:, :, w2 * 128:(w2 + 1) * 128],
                                qsrc[c2][:, :, q0:q0 + 512],
                                start=(c2 == 0), stop=(c2 == 1), perf_mode=DR)
                        if hh == 0:
                            pts[p] = p3sb.tile([128, 2, 512], E4, tag="pt",
                                               name="pt", bufs=8)
                        nc.scalar.activation(pts[p][:, hh, :], st_ps,
                                             mybir.ActivationFunctionType.Exp,
                                             bias=nshift)

                def consume_pair(p):
                    nc.tensor.matmul(rs_ps, onesd[:, :, 0:1], pts[p],
                                     start=(p == 0), stop=(p == NKP - 1),
                                     perf_mode=DR)
                    for cv in range(NCH):
                        nc.tensor.matmul(
                            ot_ps[:, cv, :],
                            v_big[:, 2 * p:2 * p + 2, cv * 128:(cv + 1) * 128],
                            pts[p], start=(p == 0), stop=(p == NKP - 1),
                            perf_mode=DR)

                for p in range(NKP + 1):
                    if p < NKP:
                        scores_pair(p)
                    if p >= 1:
                        consume_pair(p - 1)

                rs_row = p3sb.tile([1, 512], F32, tag="rs_row", name="rs_row", bufs=2)
                nc.vector.tensor_copy(rs_row, rs_ps)
                for sub in range(NSUB):
                    rt_ps = p3ps.tile([128, 1], F32, tag="sc", name="rt", bufs=3)
                    nc.tensor.transpose(
                        rt_ps, rs_row[0:1, sub * 128:(sub + 1) * 128], ones1)
                    rr = p3ac.tile([128, 1], F32, tag="rr", name="rr")
                    nc.vector.tensor_copy(rr, rt_ps)
                    nc.vector.reciprocal(rinv_t[blk * NSUB + sub], rr)
                # DVE-only evacuation: keeps P3's ScalarE queue pure-Exp
                # (no act-table reloads between blocks)
                nc.vector.tensor_copy(ots8_t[blk][0][:, :, :], ot_ps[:, 0:2, :])
                nc.vector.tensor_copy(ots8_t[blk][1][:, :, :], ot_ps[:, 2:4, :])

                for sub in range(NSUB):
                    ti = blk * NSUB + sub
                    ps_p = p3ps.tile([128, C], F32, tag="sc", name="ps_p", bufs=3)
                    for cp in range(2):
                        nc.tensor.matmul(
                            ps_p, ots8_t[blk][cp][:, :, sub * 128:(sub + 1) * 128],
                            wp8_t[cp], start=(cp == 0), stop=(cp == 1),
                            perf_mode=DR)
                    xres = p3sb.tile([128, C], F32, tag="xres", name="xres", bufs=3)
                    nc.sync.dma_start(out=xres, in_=xres_h[ti * 128:(ti + 1) * 128, :])
                    tmp = p3sb.tile([128, C], F32, tag="tmp", name="tmp", bufs=3)
                    nc.vector.scalar_tensor_tensor(
                        out=tmp, in0=ps_p, scalar=rinv_t[ti], in1=xres,
                        op0=mybir.AluOpType.mult, op1=mybir.AluOpType.add)
                    fin = p3sb.tile([128, C], F32, tag="fin", name="fin", bufs=3)
                    nc.vector.tensor_tensor(out=fin, in0=tmp, in1=bppb,
                                            op=mybir.AluOpType.add)
                    nc.sync.dma_start(out=out_h[ti * 128:(ti + 1) * 128, :], in_=fin)

    nc.compile()
    return nc


_NC_CACHE = {}


def prepare_in_maps(x, gamma, beta, wq, bq, wk, bk, wv, bv, wp, bp):
    import ml_dtypes
    BFh = ml_dtypes.bfloat16
    x = np.ascontiguousarray(np.asarray(x, dtype=np.float32))
    fold_q = bool(np.any(np.asarray(bq) != 0))
    # sel8 folds the 1/16 group average; repl8 is the binary redistribution
    mask8 = np.zeros((128, 8), np.float32)
    for p in range(128):
        mask8[p, p // GSIZE] = 1.0
    sel8 = mask8 / GSIZE
    repl8 = np.ascontiguousarray(mask8.T)
    wkf = np.asarray(wk, np.float32)
    wqf = np.asarray(wq, np.float32)
    if fold_q:
        gmat = wkf.astype(BFh)
    else:
        gmat = ((wkf @ wqf.T) * SCALE).astype(BFh)
    E4h = ml_dtypes.float8_e4m3
    wvf = np.asarray(wv, np.float32)
    wpf = np.asarray(wp, np.float32)
    # [cp, 128, i, C]: chunk-pair DoubleRow layouts for the V / out projections
    wv8 = np.ascontiguousarray(
        wvf.reshape(2, 2, 128, C).transpose(0, 2, 1, 3)).astype(E4h)
    wp8 = np.ascontiguousarray(
        wpf.reshape(2, 2, 128, C).transpose(0, 2, 1, 3)).astype(E4h)
    bpp = (np.asarray(bv, np.float32) @ wpf + np.asarray(bp, np.float32)).astype(np.float32)
    common = {
        "gmat": gmat,
        "wv8": wv8, "wp8": wp8, "bpp": bpp,
        "gamma": np.asarray(gamma, np.float32),
        "beta": np.asarray(beta, np.float32),
        "sel8": sel8, "repl8": repl8,
    }
    if fold_q:
        common["wq"] = np.asarray(wq, BFh)
        common["bq"] = np.asarray(bq, np.float32)
    xf = x.reshape(B, T, C)
    in_maps = []
    for core in range(NCORES):
        b, qoff = core // 4, (core % 4) * QS
        # rotate so this core's query strip is rows 0..QS-1 (attention and
        # group stats are permutation-invariant over tokens), then go
        # channel-major for direct DMA into the resident XT tiles
        xr = np.roll(xf[b], -qoff, axis=0)
        in_maps.append({
            **common,
            "xt": np.ascontiguousarray(xr.T.astype(BFh)),
            "xres": np.ascontiguousarray(xf[b, qoff:qoff + QS]),
        })
    return in_maps, fold_q


def kernel(x, gamma, beta, wq, bq, wk, bk, wv, bv, wp, bp):
    in_maps, fold_q = prepare_in_maps(x, gamma, beta, wq, bq, wk, bk, wv, bv, wp, bp)
    if fold_q not in _NC_CACHE:
        _NC_CACHE[fold_q] = _build(fold_q)
    nc = _NC_CACHE[fold_q]
    res = run_bass_kernel_spmd(nc, in_maps, list(range(NCORES)))
    out = np.empty((B, T, C), np.float32)
    for core in range(NCORES):
        b, qoff = core // 4, (core % 4) * QS
        out[b, qoff:qoff + QS] = res.results[core]["out"]
    return out.reshape(B, H, W, C)


# revision 31
# speedup vs baseline: 1.1663x; 1.1663x over previous
"""AttentionBlock (GroupNorm + single-head full attention + residual) on 8 trn2 cores.

Sharding: core i -> batch i//4, query strip (i%4)*1024 .. +1024. Each core
computes its batch's full K/V (duplicated across the 4 cores sharing the
batch) so no inter-core communication is needed. The host rotates each
core's copy of x so its query strip sits at token rows 0..1023 (group-norm
statistics and attention key-sums are permutation-invariant over tokens),
which lets one SPMD program serve all cores.

Changes vs the bf16 baseline (292-352us) -> 173us at full clock:
  - x arrives channel-major (host-side transpose): no PE transposes and no
    ones/Square stats matmuls. Group-norm stats: DVE bn_stats/bn_aggr for
    chunks 0-2; chunk 3 entirely on ScalarE via two Square+accum_out
    passes (sum(x) recovered from sum((x+1)^2)-sum(x^2)-n). Group combine
    and per-channel redistribution are tiny 8/128-partition matmuls
    (1/16 folded into the host sel8 matrix); rstd is plain sqrt+reciprocal
    (~4e-3, inside the fp8 error budget).
  - The Q projection is folded into the K side on the host (when bq == 0):
    Z = h @ (wk wq^T * C^-0.5), so scores S^T = Z^T . h_q use raw
    normalized h on the query side. One less projection pass.
  - The attention core runs in fp8 e4m3 with DoubleRow double-pumped
    matmuls (2 contraction chunks per instruction, issue rate measured at
    the 216ns N=512 streaming floor = 2x bf16 math): scores, exp row-sums
    and P^T.V. Z^T / V / h_q are quantized to e4m3 at PSUM evacuation.
    exp(s - 5) keeps P in e4m3 range (logits measured in [-7.5, 7.2];
    e4m3 covers [2e-3, 240]). Softmax normalization is deferred to the
    f32r projection output, so fp8 rowsum noise largely cancels.
  - P3 is software-pipelined per 256-key pair: scores/exp for pair p while
    the rowsum + P^T.V DoubleRow matmuls consume pair p-1, so the PE never
    paces on the ScalarE exp drain. PSUM: 4 ot banks + 1 rowsum + 3 score.
  - Only ONE normalized activation tensor exists: h8, e4m3 in DoubleRow
    pair layout. It feeds the V projection (full fp8 DoubleRow), the
    score query side, and the Z projection (bf16 G weights x e4m3 h at
    bf16 rate: quantizing G too would fail the gate at 3e-2, numpy-model).
    The out-projection also runs fp8 DoubleRow on e4m3-evacuated ot.
    bv@wp+bp is folded on the host. Evacuations are spread DVE/ScalarE;
    P2 ScalarE is all-Identity and P3 ScalarE is pure-Exp (act-table
    reloads cost ~1.3us each).
End-to-end absmax-relative error vs the fp32 reference: 5.9e-3 on HW
(tolerance 2e-2). HAM keep-alive matmuls hold the PE clock at 2.4GHz.
Measured 173.4us at full clock; the device alternates into a ~2.0GHz
P0 power regime under sustained load where the same NEFF reads ~204us.
"""

import numpy as np
from contextlib import ExitStack

import concourse.bass as bass
import concourse.bacc as bacc
import concourse.tile as tile
from concourse import mybir
from concourse.bass_utils import run_bass_kernel_spmd

B, H, W, C = 2, 64, 64, 512
T = H * W                 # 4096 tokens per batch
NCORES = 8
QS = 1024                 # queries per core
GROUPS, GSIZE = 32, 16    # 8 groups per 128-channel chunk
EPS = 1e-5
SCALE = float(C) ** -0.5
SHIFT = 5.0               # softmax logit shift so exp() fits e4m3
F32 = mybir.dt.float32
F32R = mybir.dt.float32r
BF16 = mybir.dt.bfloat16
E4 = mybir.dt.float8e4
DR = mybir.MatmulPerfMode.DoubleRow
NCH = C // 128            # 4 channel chunks
NW = T // 512             # 8 token windows per batch
NBLK = QS // 512          # 2 attention q-blocks of 512 queries
NSUB = 4                  # 128-query subtiles per block
NKP = T // 256            # 16 key-tile pairs per q-block


def _build(fold_q: bool):
    nc = bacc.Bacc(None, target_bir_lowering=False)

    xt_h = nc.declare_dram_parameter("xt", [C, T], BF16, isOutput=False)
    xres_h = nc.declare_dram_parameter("xres", [QS, C], F32, isOutput=False)
    g_h = nc.declare_dram_parameter("gmat", [C, C], BF16, isOutput=False)
    wv8_h = nc.declare_dram_parameter("wv8", [2, 128, 2, C], E4, isOutput=False)
    wp8_h = nc.declare_dram_parameter("wp8", [2, 128, 2, C], E4, isOutput=False)
    bpp_h = nc.declare_dram_parameter("bpp", [C], F32, isOutput=False)
    gamma_h = nc.declare_dram_parameter("gamma", [C], F32, isOutput=False)
    beta_h = nc.declare_dram_parameter("beta", [C], F32, isOutput=False)
    sel8_h = nc.declare_dram_parameter("sel8", [128, 8], F32, isOutput=False)
    repl8_h = nc.declare_dram_parameter("repl8", [8, 128], F32, isOutput=False)
    if fold_q:
        wq_h = nc.declare_dram_parameter("wq", [C, C], BF16, isOutput=False)
        bq_h = nc.declare_dram_parameter("bq", [C], F32, isOutput=False)
    out_h = nc.declare_dram_parameter("out", [QS, C], F32, isOutput=True)

    with tile.TileContext(nc) as tc, ExitStack() as ctx:
        persist = ctx.enter_context(tc.tile_pool(name="persist", bufs=1))
        small = ctx.enter_context(tc.tile_pool(name="small", bufs=1))

        bigpool = ctx.enter_context(tc.tile_pool(name="bigpool", bufs=1))
        xt_t = [bigpool.tile([128, T], BF16, tag=f"xt{j}", name=f"xt{j}") for j in range(NCH)]
        # fp8 operand tiles in DoubleRow pair layout [128, 2, ...]
        zt2 = [bigpool.tile([128, 2, T], E4, tag=f"zt{c}", name=f"zt{c}") for c in range(2)]
        h8 = [bigpool.tile([128, 2, T], E4, tag=f"h8{c}", name=f"h8{c}") for c in range(2)]
        v_big = bigpool.tile([128, T // 128, C], E4, tag="vbig", name="vbig")

        ctx2 = ExitStack()
        wpool = ctx2.enter_context(tc.tile_pool(name="wpool", bufs=1))
        g_t = [wpool.tile([128, C], BF16, tag=f"g{j}", name=f"g{j}") for j in range(NCH)]
        wv8_t = [wpool.tile([128, 2, C], E4, tag=f"wv8{j}", name=f"wv8{j}") for j in range(2)]
        wp8_t = [persist.tile([128, 2, C], E4, tag=f"wp8{j}", name=f"wp8{j}") for j in range(2)]
        if fold_q:
            wq_t = [wpool.tile([128, C], BF16, tag=f"wq{j}", name=f"wq{j}") for j in range(NCH)]
        for j in range(NCH):
            sl = slice(j * 128, (j + 1) * 128)
            nc.scalar.dma_start(out=g_t[j], in_=g_h[sl, :])
            if fold_q:
                nc.scalar.dma_start(out=wq_t[j], in_=wq_h[sl, :])
        for cp in range(2):
            nc.scalar.dma_start(out=wv8_t[cp], in_=wv8_h[cp, :, :, :])
            nc.scalar.dma_start(out=wp8_t[cp], in_=wp8_h[cp, :, :, :])

        # per-channel vectors as [128, NCH] (column j = channel chunk j)
        def vec_tile(h, name):
            t = small.tile([128, NCH], F32, tag=name)
            nc.scalar.dma_start(out=t, in_=h.rearrange("(a p) -> p a", p=128))
            return t

        gamma_sb = vec_tile(gamma_h, "gamma")
        beta_sb = vec_tile(beta_h, "beta")
        bpp_row = small.tile([1, C], F32, tag="bpprow", name="bpprow")
        nc.scalar.dma_start(out=bpp_row, in_=bpp_h.rearrange("(a c) -> a c", a=1))
        sel8 = small.tile([128, 8], F32, tag="sel8", name="sel8")
        nc.sync.dma_start(out=sel8, in_=sel8_h[:, :])
        repl8 = small.tile([8, 128], F32, tag="repl8", name="repl8")
        nc.sync.dma_start(out=repl8, in_=repl8_h[:, :])
        if fold_q:
            qts2 = [bigpool.tile([128, 2, QS], E4, tag=f"qts{c}", name=f"qts{c}") for c in range(2)]
            bq_sb = vec_tile(bq_h, "bq")
            sbq = small.tile([128, NCH], F32, tag="sbq", name="sbq")
            nc.vector.tensor_scalar_mul(sbq, bq_sb, SCALE)

        ones1 = small.tile([1, 1], F32, tag="ones1", name="ones1")
        nc.vector.memset(ones1, 1.0)
        nshift = small.tile([128, 1], F32, tag="nshift", name="nshift")
        nc.vector.memset(nshift, -SHIFT)
        onesd = small.tile([128, 2, 16], E4, tag="onesd", name="onesd")
        nc.vector.memset(onesd, 1.0)

        rinv_t = [small.tile([128, 1], F32, tag=f"rinv{s}", name=f"rinv{s}") for s in range(NSUB * NBLK)]
        scale_t = [small.tile([128, 1], F32, tag=f"gnsc{j}", name=f"gnsc{j}") for j in range(NCH)]
        bias_t = [small.tile([128, 1], F32, tag=f"gnbi{j}", name=f"gnbi{j}") for j in range(NCH)]

        # PE warm-up / keep-alive dummy matmuls (HAM unthrottle 1.2->2.4GHz)
        warm_sb = small.tile([128, 512], BF16, tag="warm_sb", name="warm_sb")
        nc.vector.memset(warm_sb, 0.0)

        ctxkeep = ExitStack()
        p1ps_keep = ctxkeep.enter_context(tc.tile_pool(name="keepps", bufs=1, space="PSUM"))

        def keepalive(n, lhs=None):
            for _ in range(n):
                kps = p1ps_keep.tile([128, 512], F32, tag="keep", name="keep", bufs=1)
                if lhs is None:
                    nc.tensor.matmul(kps, warm_sb[:, 0:128], warm_sb,
                                     start=True, stop=True)
                else:
                    nc.tensor.matmul(kps[0:1, :], lhs, warm_sb,
                                     start=True, stop=True)

        # ================= P1: stream XT, bn_stats group statistics =========
        # Per-chunk pipeline (a chunk's group scale/bias only depends on its
        # own 128 channels): half-chunk DMAs -> 3D bn_stats -> bn_aggr ->
        # tiny 8/128-partition matmuls for the 16-channel group combine and
        # per-channel redistribution.
        with tc.tile_pool(name="p1ps", bufs=1, space="PSUM") as p1ps, \
             tc.tile_pool(name="p1sb", bufs=1) as p1sb:
            keepalive(18)
            HT = T // 2
            # DMA halves interleaved so the ScalarE/reduce chunk (3) lands
            # early while the DVE bn_stats chunks stream in order
            for hf in range(2):
                for j in (3, 0, 1, 2):
                    sl = slice(hf * HT, (hf + 1) * HT)
                    nc.sync.dma_start(out=xt_t[j][:, sl],
                                      in_=xt_h[j * 128:(j + 1) * 128, sl])

            Sjs = []
            # chunk 3 entirely on ScalarE: two Square+accum passes per slice
            # (bias 0 and bias 1; one act-table segment). Algebra recovers the
            # plain sums: sum(x) = (sum((x+1)^2) - sum(x^2) - 512) / 2.
            onescol = small.tile([128, 1], F32, tag="onescol", name="onescol")
            nc.vector.memset(onescol, 1.0)
            ssq8 = p1sb.tile([128, NW], F32, tag="ssq8", name="ssq8")
            sqb8 = p1sb.tile([128, NW], F32, tag="sqb8", name="sqb8")
            for s in range(NW):
                scr = p1sb.tile([128, 512], BF16, tag="sqscr", name="sqscr", bufs=2)
                nc.scalar.activation(scr, xt_t[3][:, s * 512:(s + 1) * 512],
                                     mybir.ActivationFunctionType.Square,
                                     accum_out=ssq8[:, s:s + 1])
                scr2 = p1sb.tile([128, 512], BF16, tag="sqscr", name="sqscr", bufs=2)
                nc.scalar.activation(scr2, xt_t[3][:, s * 512:(s + 1) * 512],
                                     mybir.ActivationFunctionType.Square,
                                     bias=onescol,
                                     accum_out=sqb8[:, s:s + 1])
            # chunks 0-2: DVE bn_stats / bn_aggr
            for j in range(3):
                bns = p1sb.tile([128, NW, 6], F32, tag=f"bns{j}", name=f"bns{j}")
                for s in range(NW):
                    nc.vector.bn_stats(bns[:, s, :],
                                       xt_t[j][:, s * 512:(s + 1) * 512])
                mvj = p1sb.tile([128, 2], F32, tag=f"mv{j}", name=f"mv{j}")
                nc.vector.bn_aggr(mvj, bns)
                # Sj: col 0 = mean_c, col 1 = E[x^2]_c
                Sj = p1sb.tile([128, 2], F32, tag=f"S{j}", name=f"S{j}")
                nc.vector.tensor_copy(Sj[:, 0:1], mvj[:, 0:1])
                nc.vector.scalar_tensor_tensor(
                    out=Sj[:, 1:2], in0=mvj[:, 0:1], scalar=mvj[:, 0:1],
                    in1=mvj[:, 1:2], op0=mybir.AluOpType.mult,
                    op1=mybir.AluOpType.add)
                Sjs.append(Sj)
                if j in (0, 1, 2):
                    wj = p1sb.tile([128, 1], BF16, tag=f"warm{j}", name=f"warm{j}")
                    nc.gpsimd.tensor_copy(wj, mvj[:, 0:1])
                    keepalive(3, lhs=wj)
            S3 = p1sb.tile([128, 2], F32, tag="S3", name="S3")
            dsq = p1sb.tile([128, NW], F32, tag="dsq", name="dsq")
            nc.vector.tensor_tensor(out=dsq, in0=sqb8, in1=ssq8,
                                    op=mybir.AluOpType.subtract)
            nc.vector.tensor_reduce(out=S3[:, 0:1], in_=dsq,
                                    axis=mybir.AxisListType.X, op=mybir.AluOpType.add)
            # sum(dsq) = 2*sum(x) + T  ->  mean = (sum(dsq) - T) / (2T)
            nc.vector.tensor_scalar(out=S3[:, 0:1], in0=S3[:, 0:1],
                                    scalar1=-float(T), scalar2=0.5 / T,
                                    op0=mybir.AluOpType.add,
                                    op1=mybir.AluOpType.mult)
            nc.vector.tensor_reduce(out=S3[:, 1:2], in_=ssq8,
                                    axis=mybir.AxisListType.X, op=mybir.AluOpType.add)
            nc.vector.tensor_scalar_mul(S3[:, 1:2], S3[:, 1:2], 1.0 / T)
            Sjs.append(S3)
            w3 = p1sb.tile([128, 1], BF16, tag="warm3", name="warm3")
            nc.gpsimd.tensor_copy(w3, S3[:, 0:1])
            keepalive(3, lhs=w3)
            # pre-load the Exp activation table so the first P3 exp doesn't
            # pay the ~1.3us table switch inside the score pipeline
            expwarm = p1sb.tile([128, 1], F32, tag="expwarm", name="expwarm")
            nc.scalar.activation(expwarm, nshift,
                                 mybir.ActivationFunctionType.Exp)

            # per chunk: group-combine matmul (1/16 folded into sel8),
            # plain sqrt+reciprocal rstd (~4e-3, inside the fp8 budget),
            # redistribution matmul, then per-channel scale/bias
            for j, Sj in zip((0, 1, 2, 3), Sjs):
                g8_ps = p1ps.tile([8, 2], F32, tag="g8", name="g8", bufs=2)
                nc.tensor.matmul(g8_ps, sel8, Sj, start=True, stop=True)
                vals = p1sb.tile([8, 2], F32, tag=f"vals{j}", name=f"vals{j}")
                nc.vector.tensor_copy(vals, g8_ps)
                msq8 = p1sb.tile([8, 1], F32, tag="msq8", name="msq8")
                nc.vector.tensor_tensor(out=msq8, in0=vals[:, 0:1], in1=vals[:, 0:1],
                                        op=mybir.AluOpType.mult)
                ve = p1sb.tile([8, 1], F32, tag="ve", name="ve")
                nc.vector.tensor_tensor(out=ve, in0=vals[:, 1:2], in1=msq8,
                                        op=mybir.AluOpType.subtract)
                nc.vector.tensor_scalar_add(ve, ve, EPS)
                sd = p1sb.tile([8, 1], F32, tag="sd", name="sd")
                nc.scalar.activation(sd, ve, mybir.ActivationFunctionType.Sqrt)
                nc.vector.reciprocal(vals[:, 1:2], sd)
                b128_ps = p1ps.tile([128, 2], F32, tag="b128", name="b128", bufs=2)
                nc.tensor.matmul(b128_ps, repl8, vals, start=True, stop=True)
                bc = p1sb.tile([128, 2], F32, tag=f"bc{j}", name=f"bc{j}")
                nc.vector.tensor_copy(bc, b128_ps)
                nc.vector.tensor_tensor(out=scale_t[j], in0=bc[:, 1:2],
                                        in1=gamma_sb[:, j:j + 1],
                                        op=mybir.AluOpType.mult)
                mt = p1sb.tile([128, 1], F32, tag="mt", name="mt")
                nc.vector.tensor_tensor(out=mt, in0=bc[:, 0:1], in1=scale_t[j],
                                        op=mybir.AluOpType.mult)
                nc.vector.tensor_tensor(out=bias_t[j], in0=beta_sb[:, j:j + 1],
                                        in1=mt, op=mybir.AluOpType.subtract)

        keepalive(4)
        ctxkeep.close()

        # ================= P2: normalize window-pairs -> Z^T, V (+ Q^T) =====
        # 1024-token moving operands (bf16 moving max) into 2-bank PSUM
        # tiles: half the matmul/LDWEIGHTS/evac instruction count.
        with tc.tile_pool(name="p2ps", bufs=1, space="PSUM") as p2ps, \
             tc.tile_pool(name="p2sb", bufs=2) as p2sb:
            for wp in range(NW // 2):
                t0 = wp * 1024
                # normalize straight into the e4m3 pair-layout h8 (no bf16
                # h at all: Z runs bf16-weights x e4m3-h, V/scores run fp8)
                for j in range(3):
                    nc.vector.tensor_scalar(
                        out=h8[j // 2][:, j % 2, t0:t0 + 1024],
                        in0=xt_t[j][:, t0:t0 + 1024],
                        scalar1=scale_t[j], scalar2=bias_t[j],
                        op0=mybir.AluOpType.mult, op1=mybir.AluOpType.add)
                nc.scalar.activation(
                    h8[1][:, 1, t0:t0 + 1024], xt_t[3][:, t0:t0 + 1024],
                    mybir.ActivationFunctionType.Identity,
                    bias=bias_t[3], scale=scale_t[3])
                for ck in range(NCH):
                    ps2 = p2ps.tile([128, 2, 512], F32, tag="zp", name="zp", bufs=2)
                    for hh in range(2):
                        hs = t0 + hh * 512
                        for ci in range(NCH):
                            nc.tensor.matmul(
                                ps2[:, hh, :], g_t[ci][:, ck * 128:(ck + 1) * 128],
                                h8[ci // 2][:, ci % 2, hs:hs + 512],
                                start=(ci == 0), stop=(ci == NCH - 1))
                    nc.vector.tensor_copy(zt2[ck // 2][:, ck % 2, t0:t0 + 1024], ps2)
                for m in range(4):
                    ps2 = p2ps.tile([128, 2, 512], F32, tag="vp", name="vp", bufs=2)
                    for hh in range(2):
                        tb = wp * 8 + 2 * m + hh
                        for cp in range(2):
                            nc.tensor.matmul(
                                ps2[:, hh, :],
                                h8[cp][:, :, tb * 128:(tb + 1) * 128],
                                wv8_t[cp], start=(cp == 0), stop=(cp == 1),
                                perf_mode=DR)
                    nc.scalar.activation(
                        v_big[:, wp * 8 + 2 * m:wp * 8 + 2 * m + 2, :], ps2,
                        mybir.ActivationFunctionType.Identity)
                if wp == 0 and fold_q:
                    for cq in range(NCH):
                        ps2 = p2ps.tile([128, 2, 512], F32, tag="zp", name="zp", bufs=2)
                        for hh in range(2):
                            hs = t0 + hh * 512
                            for ci in range(NCH):
                                nc.tensor.matmul(
                                    ps2[:, hh, :], wq_t[ci][:, cq * 128:(cq + 1) * 128],
                                    h8[ci // 2][:, ci % 2, hs:hs + 512],
                                    start=(ci == 0), stop=(ci == NCH - 1))
                        nc.scalar.activation(
                            qts2[cq // 2][:, cq % 2, :], ps2,
                            mybir.ActivationFunctionType.Identity,
                            bias=sbq[:, cq:cq + 1], scale=SCALE)
        ctx2.close()

        # ================= P3: fp8 DoubleRow attention =======================
        otspool = ctx.enter_context(tc.tile_pool(name="otspool", bufs=1))
        ots8_t = [[otspool.tile([128, 2, 512], E4, tag=f"ots{b}_{cp}", name=f"ots{b}_{cp}")
                   for cp in range(2)] for b in range(NBLK)]
        with tc.tile_pool(name="p3ps", bufs=1, space="PSUM") as p3ps, \
             tc.tile_pool(name="p3ot", bufs=1, space="PSUM") as p3ot, \
             tc.tile_pool(name="p3sb", bufs=1) as p3sb, \
             tc.tile_pool(name="p3ac", bufs=4) as p3ac:
            # bias vector bp' = bv @ wp + bp (host-folded), broadcast
            bppb = p3sb.tile([128, C], F32, tag="bppb", name="bppb")
            nc.gpsimd.partition_broadcast(bppb, bpp_row[0:1, :])

            for blk in range(NBLK):
                q0 = blk * 512
                ot_ps = p3ot.tile([128, NCH, 512], F32, tag="ot", name="ot", bufs=1)
                rs_ps = p3ot.tile([1, 512], F32, tag="rsum", name="rsum", bufs=1)
                pts = [None] * NKP

                # software-pipelined: scores/exp for key-pair p while the
                # rowsum + P^T.V matmuls consume pair p-1 (PE stays busy
                # instead of pacing on the ScalarE exp drain)
                def scores_pair(p):
                    for hh in range(2):
                        w2 = 2 * p + hh
                        st_ps = p3ps.tile([128, 512], F32, tag="sc", name="st_ps", bufs=3)
                        qsrc = qts2 if fold_q else h8
                        for c2 in range(2):
                            nc.tensor.matmul(
                                st_ps, zt2[c2][:, :, w2 * 128:(w2 + 1) * 128],
                                qsrc[c2][:, :, q0:q0 + 512],
                                start=(c2 == 0), stop=(c2 == 1), perf_mode=DR)
                        if hh == 0:
                            pts[p] = p3sb.tile([128, 2, 512], E4, tag="pt",
                                               name="pt", bufs=8)
                        nc.scalar.activation(pts[p][:, hh, :], st_ps,
                                             mybir.ActivationFunctionType.Exp,
                                             bias=nshift)

                def consume_pair(p):
                    nc.tensor.matmul(rs_ps, onesd[:, :, 0:1], pts[p],
                                     start=(p == 0), stop=(p == NKP - 1),
                                     perf_mode=DR)
                    for cv in range(NCH):
                        nc.tensor.matmul(
                            ot_ps[:, cv, :],
                            v_big[:, 2 * p:2 * p + 2, cv * 128:(cv + 1) * 128],
                            pts[p], start=(p == 0), stop=(p == NKP - 1),
                            perf_mode=DR)

                for p in range(NKP + 1):
                    if p < NKP:
                        scores_pair(p)
                    if p >= 1:
                        consume_pair(p - 1)

                rs_row = p3sb.tile([1, 512], F32, tag="rs_row", name="rs_row", bufs=2)
                nc.vector.tensor_copy(rs_row, rs_ps)
                for sub in range(NSUB):
                    rt_ps = p3ps.tile([128, 1], F32, tag="sc", name="rt", bufs=3)
                    nc.tensor.transpose(
                        rt_ps, rs_row[0:1, sub * 128:(sub + 1) * 128], ones1)
                    rr = p3ac.tile([128, 1], F32, tag="rr", name="rr")
                    nc.vector.tensor_copy(rr, rt_ps)
                    nc.vector.reciprocal(rinv_t[blk * NSUB + sub], rr)
                # DVE-only evacuation: keeps P3's ScalarE queue pure-Exp
                # (no act-table reloads between blocks)
                nc.vector.tensor_copy(ots8_t[blk][0][:, :, :], ot_ps[:, 0:2, :])
                nc.vector.tensor_copy(ots8_t[blk][1][:, :, :], ot_ps[:, 2:4, :])

                for sub in range(NSUB):
                    ti = blk * NSUB + sub
                    ps_p = p3ps.tile([128, C], F32, tag="sc", name="ps_p", bufs=3)
                    for cp in range(2):
                        nc.tensor.matmul(
                            ps_p, ots8_t[blk][cp][:, :, sub * 128:(sub + 1) * 128],
                            wp8_t[cp], start=(cp == 0), stop=(cp == 1),
                            perf_mode=DR)
                    xres = p3sb.tile([128, C], F32, tag="xres", name="xres", bufs=3)
                    nc.sync.dma_start(out=xres, in_=xres_h[ti * 128:(ti + 1) * 128, :])
                    tmp = p3sb.tile([128, C], F32, tag="tmp", name="tmp", bufs=3)
                    nc.vector.scalar_tensor_tensor(
                        out=tmp, in0=ps_p, scalar=rinv_t[ti], in1=xres,
                        op0=mybir.AluOpType.mult, op1=mybir.AluOpType.add)
                    fin = p3sb.tile([128, C], F32, tag="fin", name="fin", bufs=3)
                    nc.vector.tensor_tensor(out=fin, in0=tmp, in1=bppb,
                                            op=mybir.AluOpType.add)
                    nc.sync.dma_start(out=out_h[ti * 128:(ti + 1) * 128, :], in_=fin)

    nc.compile()
    return nc


_NC_CACHE = {}


def prepare_in_maps(x, gamma, beta, wq, bq, wk, bk, wv, bv, wp, bp):
    import ml_dtypes
    BFh = ml_dtypes.bfloat16
    x = np.ascontiguousarray(np.asarray(x, dtype=np.float32))
    fold_q = bool(np.any(np.asarray(bq) != 0))
    # sel8 folds the 1/16 group average; repl8 is the binary redistribution
    mask8 = np.zeros((128, 8), np.float32)
    for p in range(128):
        mask8[p, p // GSIZE] = 1.0
    sel8 = mask8 / GSIZE
    repl8 = np.ascontiguousarray(mask8.T)
    wkf = np.asarray(wk, np.float32)
    wqf = np.asarray(wq, np.float32)
    if fold_q:
        gmat = wkf.astype(BFh)
    else:
        gmat = ((wkf @ wqf.T) * SCALE).astype(BFh)
    E4h = ml_dtypes.float8_e4m3
    wvf = np.asarray(wv, np.float32)
    wpf = np.asarray(wp, np.float32)
    # [cp, 128, i, C]: chunk-pair DoubleRow layouts for the V / out projections
    wv8 = np.ascontiguousarray(
        wvf.reshape(2, 2, 128, C).transpose(0, 2, 1, 3)).astype(E4h)
    wp8 = np.ascontiguousarray(
        wpf.reshape(2, 2, 128, C).transpose(0, 2, 1, 3)).astype(E4h)
    bpp = (np.asarray(bv, np.float32) @ wpf + np.asarray(bp, np.float32)).astype(np.float32)
    common = {
        "gmat": gmat,
        "wv8": wv8, "wp8": wp8, "bpp": bpp,
        "gamma": np.asarray(gamma, np.float32),
        "beta": np.asarray(beta, np.float32),
        "sel8": sel8, "repl8": repl8,
    }
    if fold_q:
        common["wq"] = np.asarray(wq, BFh)
        common["bq"] = np.asarray(bq, np.float32)
    xf = x.reshape(B, T, C)
    in_maps = []
    for core in range(NCORES):
        b, qoff = core // 4, (core % 4) * QS
        # rotate so this core's query strip is rows 0..QS-1 (attention and
        # group stats are permutation-invariant over tokens), then go
        # channel-major for direct DMA into the resident XT tiles
        xr = np.roll(xf[b], -qoff, axis=0)
        in_maps.append({
            **common,
            "xt": np.ascontiguousarray(xr.T.astype(BFh)),
            "xres": np.ascontiguousarray(xf[b, qoff:qoff + QS]),
        })
    return in_maps, fold_q


def kernel(x, gamma, beta, wq, bq, wk, bk, wv, bv, wp, bp):
    in_maps, fold_q = prepare_in_maps(x, gamma, beta, wq, bq, wk, bk, wv, bv, wp, bp)
    if fold_q not in _NC_CACHE:
        _NC_CACHE[fold_q] = _build(fold_q)
    nc = _NC_CACHE[fold_q]
    res = run_bass_kernel_spmd(nc, in_maps, list(range(NCORES)))
    out = np.empty((B, T, C), np.float32)
    for core in range(NCORES):
        b, qoff = core // 4, (core % 4) * QS
        out[b, qoff:qoff + QS] = res.results[core]["out"]
    return out.reshape(B, H, W, C)


# revision 33
# speedup vs baseline: 1.2396x; 1.0629x over previous
"""AttentionBlock (GroupNorm + single-head full attention + residual) on 8 trn2 cores.

Sharding: core i -> batch i//4, query strip (i%4)*1024 .. +1024. Each core
computes its batch's full K/V (duplicated across the 4 cores sharing the
batch) so no inter-core communication is needed. The host rotates each
core's copy of x so its query strip sits at token rows 0..1023 (group-norm
statistics and attention key-sums are permutation-invariant over tokens),
which lets one SPMD program serve all cores.

Changes vs the bf16 baseline (292-352us) -> 173us at full clock:
  - x arrives channel-major (host-side transpose): no PE transposes and no
    ones/Square stats matmuls. Group-norm stats: DVE bn_stats/bn_aggr for
    chunks 0-2; chunk 3 entirely on ScalarE via two Square+accum_out
    passes (sum(x) recovered from sum((x+1)^2)-sum(x^2)-n). Group combine
    and per-channel redistribution are tiny 8/128-partition matmuls
    (1/16 folded into the host sel8 matrix); rstd is plain sqrt+reciprocal
    (~4e-3, inside the fp8 error budget).
  - The Q projection is folded into the K side on the host (when bq == 0):
    Z = h @ (wk wq^T * C^-0.5), so scores S^T = Z^T . h_q use raw
    normalized h on the query side. One less projection pass.
  - The attention core runs in fp8 e4m3 with DoubleRow double-pumped
    matmuls (2 contraction chunks per instruction, issue rate measured at
    the 216ns N=512 streaming floor = 2x bf16 math): scores, exp row-sums
    and P^T.V. Z^T / V / h_q are quantized to e4m3 at PSUM evacuation.
    exp(s - 5) keeps P in e4m3 range (logits measured in [-7.5, 7.2];
    e4m3 covers [2e-3, 240]). Softmax normalization is deferred to the
    f32r projection output, so fp8 rowsum noise largely cancels.
  - P3 is software-pipelined per 256-key pair: scores/exp for pair p while
    the rowsum + P^T.V DoubleRow matmuls consume pair p-1, so the PE never
    paces on the ScalarE exp drain. PSUM: 4 ot banks + 1 rowsum + 3 score.
  - Only ONE normalized activation tensor exists: h8, e4m3 in DoubleRow
    pair layout. It feeds the V projection (full fp8 DoubleRow), the
    score query side, and the Z projection (bf16 G weights x e4m3 h at
    bf16 rate: quantizing G too would fail the gate at 3e-2, numpy-model).
    The out-projection also runs fp8 DoubleRow on e4m3-evacuated ot.
    bv@wp+bp is folded on the host. Evacuations are spread DVE/ScalarE;
    P2 ScalarE is all-Identity and P3 ScalarE is pure-Exp (act-table
    reloads cost ~1.3us each).
End-to-end absmax-relative error vs the fp32 reference: 5.9e-3 on HW
(tolerance 2e-2). HAM keep-alive matmuls hold the PE clock at 2.4GHz.
Measured 173.4us at full clock; the device alternates into a ~2.0GHz
P0 power regime under sustained load where the same NEFF reads ~204us.
"""

import numpy as np
from contextlib import ExitStack

import concourse.bass as bass
import concourse.bacc as bacc
import concourse.tile as tile
from concourse import mybir
from concourse.bass_utils import run_bass_kernel_spmd

B, H, W, C = 2, 64, 64, 512
T = H * W                 # 4096 tokens per batch
NCORES = 8
QS = 1024                 # queries per core
GROUPS, GSIZE = 32, 16    # 8 groups per 128-channel chunk
EPS = 1e-5
SCALE = float(C) ** -0.5
SHIFT = 5.0               # softmax logit shift so exp() fits e4m3
F32 = mybir.dt.float32
F32R = mybir.dt.float32r
BF16 = mybir.dt.bfloat16
E4 = mybir.dt.float8e4
DR = mybir.MatmulPerfMode.DoubleRow
NCH = C // 128            # 4 channel chunks
NW = T // 512             # 8 token windows per batch
NBLK = QS // 512          # 2 attention q-blocks of 512 queries
NSUB = 4                  # 128-query subtiles per block
NKP = T // 256            # 16 key-tile pairs per q-block


def _build(fold_q: bool):
    nc = bacc.Bacc(None, target_bir_lowering=False)

    xt_h = nc.declare_dram_parameter("xt", [C, T], BF16, isOutput=False)
    xres_h = nc.declare_dram_parameter("xres", [QS, C], F32, isOutput=False)
    g_h = nc.declare_dram_parameter("gmat", [C, C], BF16, isOutput=False)
    wv8_h = nc.declare_dram_parameter("wv8", [2, 128, 2, C], E4, isOutput=False)
    wp8_h = nc.declare_dram_parameter("wp8", [2, 128, 2, C], E4, isOutput=False)
    bpp_h = nc.declare_dram_parameter("bpp", [C], F32, isOutput=False)
    gamma_h = nc.declare_dram_parameter("gamma", [C], F32, isOutput=False)
    beta_h = nc.declare_dram_parameter("beta", [C], F32, isOutput=False)
    sel8_h = nc.declare_dram_parameter("sel8", [128, 8], F32, isOutput=False)
    repl8_h = nc.declare_dram_parameter("repl8", [8, 128], F32, isOutput=False)
    bqwk_h = nc.declare_dram_parameter("bqwk", [C], F32, isOutput=False)
    out_h = nc.declare_dram_parameter("out", [QS, C], F32, isOutput=True)

    with tile.TileContext(nc) as tc, ExitStack() as ctx:
        persist = ctx.enter_context(tc.tile_pool(name="persist", bufs=1))
        small = ctx.enter_context(tc.tile_pool(name="small", bufs=1))

        bigpool = ctx.enter_context(tc.tile_pool(name="bigpool", bufs=1))
        xt_t = [bigpool.tile([128, T], BF16, tag=f"xt{j}", name=f"xt{j}") for j in range(NCH)]
        # fp8 operand tiles in DoubleRow pair layout [128, 2, ...]
        y8 = [bigpool.tile([128, 2, QS], E4, tag=f"y8{c}", name=f"y8{c}") for c in range(2)]
        h8 = [bigpool.tile([128, 2, T], E4, tag=f"h8{c}", name=f"h8{c}") for c in range(2)]
        v_big = bigpool.tile([128, T // 128, C], E4, tag="vbig", name="vbig")

        ctx2 = ExitStack()
        wpool = ctx2.enter_context(tc.tile_pool(name="wpool", bufs=1))
        g_t = [wpool.tile([128, C], BF16, tag=f"g{j}", name=f"g{j}") for j in range(NCH)]
        wv8_t = [wpool.tile([128, 2, C], E4, tag=f"wv8{j}", name=f"wv8{j}") for j in range(2)]
        wp8_t = [persist.tile([128, 2, C], E4, tag=f"wp8{j}", name=f"wp8{j}") for j in range(2)]
        for j in range(NCH):
            sl = slice(j * 128, (j + 1) * 128)
            nc.scalar.dma_start(out=g_t[j], in_=g_h[sl, :])
        for cp in range(2):
            nc.scalar.dma_start(out=wv8_t[cp], in_=wv8_h[cp, :, :, :])
            nc.scalar.dma_start(out=wp8_t[cp], in_=wp8_h[cp, :, :, :])

        # per-channel vectors as [128, NCH] (column j = channel chunk j)
        def vec_tile(h, name):
            t = small.tile([128, NCH], F32, tag=name)
            nc.scalar.dma_start(out=t, in_=h.rearrange("(a p) -> p a", p=128))
            return t

        gamma_sb = vec_tile(gamma_h, "gamma")
        beta_sb = vec_tile(beta_h, "beta")
        bpp_row = small.tile([1, C], F32, tag="bpprow", name="bpprow")
        nc.scalar.dma_start(out=bpp_row, in_=bpp_h.rearrange("(a c) -> a c", a=1))
        sel8 = small.tile([128, 8], F32, tag="sel8", name="sel8")
        nc.sync.dma_start(out=sel8, in_=sel8_h[:, :])
        repl8 = small.tile([8, 128], F32, tag="repl8", name="repl8")
        nc.sync.dma_start(out=repl8, in_=repl8_h[:, :])
        bqwk_sb = vec_tile(bqwk_h, "bqwk")

        ones1 = small.tile([1, 1], F32, tag="ones1", name="ones1")
        nc.vector.memset(ones1, 1.0)
        nshift = small.tile([128, 1], F32, tag="nshift", name="nshift")
        nc.vector.memset(nshift, -SHIFT)
        onesd = small.tile([128, 2, 16], E4, tag="onesd", name="onesd")
        nc.vector.memset(onesd, 1.0)

        rinv_t = [small.tile([128, 1], F32, tag=f"rinv{s}", name=f"rinv{s}") for s in range(NSUB * NBLK)]
        scale_t = [small.tile([128, 1], F32, tag=f"gnsc{j}", name=f"gnsc{j}") for j in range(NCH)]
        bias_t = [small.tile([128, 1], F32, tag=f"gnbi{j}", name=f"gnbi{j}") for j in range(NCH)]

        # PE warm-up / keep-alive dummy matmuls (HAM unthrottle 1.2->2.4GHz)
        warm_sb = small.tile([128, 512], BF16, tag="warm_sb", name="warm_sb")
        nc.vector.memset(warm_sb, 0.0)

        ctxkeep = ExitStack()
        p1ps_keep = ctxkeep.enter_context(tc.tile_pool(name="keepps", bufs=1, space="PSUM"))

        def keepalive(n, lhs=None):
            for _ in range(n):
                kps = p1ps_keep.tile([128, 512], F32, tag="keep", name="keep", bufs=1)
                if lhs is None:
                    nc.tensor.matmul(kps, warm_sb[:, 0:128], warm_sb,
                                     start=True, stop=True)
                else:
                    nc.tensor.matmul(kps[0:1, :], lhs, warm_sb,
                                     start=True, stop=True)

        # ================= P1: stream XT, bn_stats group statistics =========
        # Per-chunk pipeline (a chunk's group scale/bias only depends on its
        # own 128 channels): half-chunk DMAs -> 3D bn_stats -> bn_aggr ->
        # tiny 8/128-partition matmuls for the 16-channel group combine and
        # per-channel redistribution.
        with tc.tile_pool(name="p1ps", bufs=1, space="PSUM") as p1ps, \
             tc.tile_pool(name="p1sb", bufs=1) as p1sb:
            keepalive(18)
            HT = T // 2
            # DMA halves interleaved so the ScalarE/reduce chunk (3) lands
            # early while the DVE bn_stats chunks stream in order
            for hf in range(2):
                for j in (3, 0, 1, 2):
                    sl = slice(hf * HT, (hf + 1) * HT)
                    nc.sync.dma_start(out=xt_t[j][:, sl],
                                      in_=xt_h[j * 128:(j + 1) * 128, sl])

            Sjs = []
            # chunk 3 entirely on ScalarE: two Square+accum passes per slice
            # (bias 0 and bias 1; one act-table segment). Algebra recovers the
            # plain sums: sum(x) = (sum((x+1)^2) - sum(x^2) - 512) / 2.
            onescol = small.tile([128, 1], F32, tag="onescol", name="onescol")
            nc.vector.memset(onescol, 1.0)
            ssq8 = p1sb.tile([128, NW], F32, tag="ssq8", name="ssq8")
            sqb8 = p1sb.tile([128, NW], F32, tag="sqb8", name="sqb8")
            for s in range(NW):
                scr = p1sb.tile([128, 512], BF16, tag="sqscr", name="sqscr", bufs=2)
                nc.scalar.activation(scr, xt_t[3][:, s * 512:(s + 1) * 512],
                                     mybir.ActivationFunctionType.Square,
                                     accum_out=ssq8[:, s:s + 1])
                scr2 = p1sb.tile([128, 512], BF16, tag="sqscr", name="sqscr", bufs=2)
                nc.scalar.activation(scr2, xt_t[3][:, s * 512:(s + 1) * 512],
                                     mybir.ActivationFunctionType.Square,
                                     bias=onescol,
                                     accum_out=sqb8[:, s:s + 1])
            # chunks 0-2: DVE bn_stats / bn_aggr
            for j in range(3):
                bns = p1sb.tile([128, NW, 6], F32, tag=f"bns{j}", name=f"bns{j}")
                for s in range(NW):
                    nc.vector.bn_stats(bns[:, s, :],
                                       xt_t[j][:, s * 512:(s + 1) * 512])
                mvj = p1sb.tile([128, 2], F32, tag=f"mv{j}", name=f"mv{j}")
                nc.vector.bn_aggr(mvj, bns)
                # Sj: col 0 = mean_c, col 1 = E[x^2]_c
                Sj = p1sb.tile([128, 2], F32, tag=f"S{j}", name=f"S{j}")
                nc.vector.tensor_copy(Sj[:, 0:1], mvj[:, 0:1])
                nc.vector.scalar_tensor_tensor(
                    out=Sj[:, 1:2], in0=mvj[:, 0:1], scalar=mvj[:, 0:1],
                    in1=mvj[:, 1:2], op0=mybir.AluOpType.mult,
                    op1=mybir.AluOpType.add)
                Sjs.append(Sj)
                if j in (0, 1, 2):
                    wj = p1sb.tile([128, 1], BF16, tag=f"warm{j}", name=f"warm{j}")
                    nc.gpsimd.tensor_copy(wj, mvj[:, 0:1])
                    keepalive(3, lhs=wj)
            S3 = p1sb.tile([128, 2], F32, tag="S3", name="S3")
            dsq = p1sb.tile([128, NW], F32, tag="dsq", name="dsq")
            nc.vector.tensor_tensor(out=dsq, in0=sqb8, in1=ssq8,
                                    op=mybir.AluOpType.subtract)
            nc.vector.tensor_reduce(out=S3[:, 0:1], in_=dsq,
                                    axis=mybir.AxisListType.X, op=mybir.AluOpType.add)
            # sum(dsq) = 2*sum(x) + T  ->  mean = (sum(dsq) - T) / (2T)
            nc.vector.tensor_scalar(out=S3[:, 0:1], in0=S3[:, 0:1],
                                    scalar1=-float(T), scalar2=0.5 / T,
                                    op0=mybir.AluOpType.add,
                                    op1=mybir.AluOpType.mult)
            nc.vector.tensor_reduce(out=S3[:, 1:2], in_=ssq8,
                                    axis=mybir.AxisListType.X, op=mybir.AluOpType.add)
            nc.vector.tensor_scalar_mul(S3[:, 1:2], S3[:, 1:2], 1.0 / T)
            Sjs.append(S3)
            w3 = p1sb.tile([128, 1], BF16, tag="warm3", name="warm3")
            nc.gpsimd.tensor_copy(w3, S3[:, 0:1])
            keepalive(3, lhs=w3)
            # pre-load the Exp activation table so the first P3 exp doesn't
            # pay the ~1.3us table switch inside the score pipeline
            expwarm = p1sb.tile([128, 1], F32, tag="expwarm", name="expwarm")
            nc.scalar.activation(expwarm, nshift,
                                 mybir.ActivationFunctionType.Exp)

            # per chunk: group-combine matmul (1/16 folded into sel8),
            # plain sqrt+reciprocal rstd (~4e-3, inside the fp8 budget),
            # redistribution matmul, then per-channel scale/bias
            for j, Sj in zip((0, 1, 2, 3), Sjs):
                g8_ps = p1ps.tile([8, 2], F32, tag="g8", name="g8", bufs=2)
                nc.tensor.matmul(g8_ps, sel8, Sj, start=True, stop=True)
                vals = p1sb.tile([8, 2], F32, tag=f"vals{j}", name=f"vals{j}")
                nc.vector.tensor_copy(vals, g8_ps)
                msq8 = p1sb.tile([8, 1], F32, tag="msq8", name="msq8")
                nc.vector.tensor_tensor(out=msq8, in0=vals[:, 0:1], in1=vals[:, 0:1],
                                        op=mybir.AluOpType.mult)
                ve = p1sb.tile([8, 1], F32, tag="ve", name="ve")
                nc.vector.tensor_tensor(out=ve, in0=vals[:, 1:2], in1=msq8,
                                        op=mybir.AluOpType.subtract)
                nc.vector.tensor_scalar_add(ve, ve, EPS)
                sd = p1sb.tile([8, 1], F32, tag="sd", name="sd")
                nc.scalar.activation(sd, ve, mybir.ActivationFunctionType.Sqrt)
                nc.vector.reciprocal(vals[:, 1:2], sd)
                b128_ps = p1ps.tile([128, 2], F32, tag="b128", name="b128", bufs=2)
                nc.tensor.matmul(b128_ps, repl8, vals, start=True, stop=True)
                bc = p1sb.tile([128, 2], F32, tag=f"bc{j}", name=f"bc{j}")
                nc.vector.tensor_copy(bc, b128_ps)
                nc.vector.tensor_tensor(out=scale_t[j], in0=bc[:, 1:2],
                                        in1=gamma_sb[:, j:j + 1],
                                        op=mybir.AluOpType.mult)
                mt = p1sb.tile([128, 1], F32, tag="mt", name="mt")
                nc.vector.tensor_tensor(out=mt, in0=bc[:, 0:1], in1=scale_t[j],
                                        op=mybir.AluOpType.mult)
                nc.vector.tensor_tensor(out=bias_t[j], in0=beta_sb[:, j:j + 1],
                                        in1=mt, op=mybir.AluOpType.subtract)

        keepalive(4)
        ctxkeep.close()

        # ================= P2: normalize window-pairs -> Z^T, V (+ Q^T) =====
        # 1024-token moving operands (bf16 moving max) into 2-bank PSUM
        # tiles: half the matmul/LDWEIGHTS/evac instruction count.
        with tc.tile_pool(name="p2ps", bufs=1, space="PSUM") as p2ps, \
             tc.tile_pool(name="p2sb", bufs=2) as p2sb:
            for wp in range(NW // 2):
                t0 = wp * 1024
                # normalize straight into the e4m3 pair-layout h8 (only
                # e4m3 h exists; Y runs bf16-weights x e4m3-h at bf16 rate)
                for j in range(2):
                    nc.vector.tensor_scalar(
                        out=h8[j // 2][:, j % 2, t0:t0 + 1024],
                        in0=xt_t[j][:, t0:t0 + 1024],
                        scalar1=scale_t[j], scalar2=bias_t[j],
                        op0=mybir.AluOpType.mult, op1=mybir.AluOpType.add)
                for j in range(2, NCH):
                    nc.scalar.activation(
                        h8[j // 2][:, j % 2, t0:t0 + 1024],
                        xt_t[j][:, t0:t0 + 1024],
                        mybir.ActivationFunctionType.Identity,
                        bias=bias_t[j], scale=scale_t[j])
                if wp == 0:
                    # Y = h_q (wq wk^T) * scale + bq wk^T : only the 1024
                    # query rows ever need projecting (key side stays raw h8)
                    for yc in range(NCH):
                        ps2 = p2ps.tile([128, 2, 512], F32, tag="zp", name="zp", bufs=2)
                        for hh in range(2):
                            hs = t0 + hh * 512
                            for ci in range(NCH):
                                nc.tensor.matmul(
                                    ps2[:, hh, :], g_t[ci][:, yc * 128:(yc + 1) * 128],
                                    h8[ci // 2][:, ci % 2, hs:hs + 512],
                                    start=(ci == 0), stop=(ci == NCH - 1))
                        nc.vector.tensor_scalar_add(
                            y8[yc // 2][:, yc % 2, :], ps2, bqwk_sb[:, yc:yc + 1])
                for m in range(4):
                    ps2 = p2ps.tile([128, 2, 512], F32, tag="vp", name="vp", bufs=2)
                    for hh in range(2):
                        tb = wp * 8 + 2 * m + hh
                        for cp in range(2):
                            nc.tensor.matmul(
                                ps2[:, hh, :],
                                h8[cp][:, :, tb * 128:(tb + 1) * 128],
                                wv8_t[cp], start=(cp == 0), stop=(cp == 1),
                                perf_mode=DR)
                    if m < 2:
                        nc.vector.tensor_copy(
                            v_big[:, wp * 8 + 2 * m:wp * 8 + 2 * m + 2, :], ps2)
                    else:
                        nc.scalar.activation(
                            v_big[:, wp * 8 + 2 * m:wp * 8 + 2 * m + 2, :], ps2,
                            mybir.ActivationFunctionType.Identity)
        ctx2.close()

        # ================= P3: fp8 DoubleRow attention =======================
        otspool = ctx.enter_context(tc.tile_pool(name="otspool", bufs=1))
        ots8_t = [[otspool.tile([128, 2, 512], E4, tag=f"ots{b}_{cp}", name=f"ots{b}_{cp}")
                   for cp in range(2)] for b in range(NBLK)]
        with tc.tile_pool(name="p3ps", bufs=1, space="PSUM") as p3ps, \
             tc.tile_pool(name="p3ot", bufs=1, space="PSUM") as p3ot, \
             tc.tile_pool(name="p3sb", bufs=1) as p3sb, \
             tc.tile_pool(name="p3ac", bufs=4) as p3ac:
            # bias vector bp' = bv @ wp + bp (host-folded), broadcast
            bppb = p3sb.tile([128, C], F32, tag="bppb", name="bppb")
            nc.gpsimd.partition_broadcast(bppb, bpp_row[0:1, :])

            for blk in range(NBLK):
                q0 = blk * 512
                ot_ps = p3ot.tile([128, NCH, 512], F32, tag="ot", name="ot", bufs=1)
                rs_ps = p3ot.tile([1, 512], F32, tag="rsum", name="rsum", bufs=1)
                pts = [None] * NKP

                # software-pipelined: scores/exp for key-pair p while the
                # rowsum + P^T.V matmuls consume pair p-1 (PE stays busy
                # instead of pacing on the ScalarE exp drain)
                def scores_pair(p):
                    for hh in range(2):
                        w2 = 2 * p + hh
                        st_ps = p3ps.tile([128, 512], F32, tag="sc", name="st_ps", bufs=3)
                        for c2 in range(2):
                            nc.tensor.matmul(
                                st_ps, h8[c2][:, :, w2 * 128:(w2 + 1) * 128],
                                y8[c2][:, :, q0:q0 + 512],
                                start=(c2 == 0), stop=(c2 == 1), perf_mode=DR)
                        if hh == 0:
                            pts[p] = p3sb.tile([128, 2, 512], E4, tag="pt",
                                               name="pt", bufs=8)
                        nc.scalar.activation(pts[p][:, hh, :], st_ps,
                                             mybir.ActivationFunctionType.Exp,
                                             bias=nshift)

                def consume_pair(p):
                    nc.tensor.matmul(rs_ps, onesd[:, :, 0:1], pts[p],
                                     start=(p == 0), stop=(p == NKP - 1),
                                     perf_mode=DR)
                    for cv in range(NCH):
                        nc.tensor.matmul(
                            ot_ps[:, cv, :],
                            v_big[:, 2 * p:2 * p + 2, cv * 128:(cv + 1) * 128],
                            pts[p], start=(p == 0), stop=(p == NKP - 1),
                            perf_mode=DR)

                for p in range(NKP + 1):
                    if p < NKP:
                        scores_pair(p)
                    if p >= 1:
                        consume_pair(p - 1)

                rs_row = p3sb.tile([1, 512], F32, tag="rs_row", name="rs_row", bufs=2)
                nc.vector.tensor_copy(rs_row, rs_ps)
                for sub in range(NSUB):
                    rt_ps = p3ps.tile([128, 1], F32, tag="sc", name="rt", bufs=3)
                    nc.tensor.transpose(
                        rt_ps, rs_row[0:1, sub * 128:(sub + 1) * 128], ones1)
                    rr = p3ac.tile([128, 1], F32, tag="rr", name="rr")
                    nc.vector.tensor_copy(rr, rt_ps)
                    nc.vector.reciprocal(rinv_t[blk * NSUB + sub], rr)
                # DVE-only evacuation: keeps P3's ScalarE queue pure-Exp
                # (no act-table reloads between blocks)
                nc.vector.tensor_copy(ots8_t[blk][0][:, :, :], ot_ps[:, 0:2, :])
                nc.vector.tensor_copy(ots8_t[blk][1][:, :, :], ot_ps[:, 2:4, :])

                for sub in range(NSUB):
                    ti = blk * NSUB + sub
                    ps_p = p3ps.tile([128, C], F32, tag="sc", name="ps_p", bufs=3)
                    for cp in range(2):
                        nc.tensor.matmul(
                            ps_p, ots8_t[blk][cp][:, :, sub * 128:(sub + 1) * 128],
                            wp8_t[cp], start=(cp == 0), stop=(cp == 1),
                            perf_mode=DR)
                    xres = p3sb.tile([128, C], F32, tag="xres", name="xres", bufs=3)
                    nc.sync.dma_start(out=xres, in_=xres_h[ti * 128:(ti + 1) * 128, :])
                    tmp = p3sb.tile([128, C], F32, tag="tmp", name="tmp", bufs=3)
                    nc.vector.scalar_tensor_tensor(
                        out=tmp, in0=ps_p, scalar=rinv_t[ti], in1=xres,
                        op0=mybir.AluOpType.mult, op1=mybir.AluOpType.add)
                    fin = p3sb.tile([128, C], F32, tag="fin", name="fin", bufs=3)
                    nc.vector.tensor_tensor(out=fin, in0=tmp, in1=bppb,
                                            op=mybir.AluOpType.add)
                    nc.sync.dma_start(out=out_h[ti * 128:(ti + 1) * 128, :], in_=fin)

    nc.compile()
    return nc


_NC_CACHE = {}


def prepare_in_maps(x, gamma, beta, wq, bq, wk, bk, wv, bv, wp, bp):
    import ml_dtypes
    BFh = ml_dtypes.bfloat16
    x = np.ascontiguousarray(np.asarray(x, dtype=np.float32))
    fold_q = False
    # sel8 folds the 1/16 group average; repl8 is the binary redistribution
    mask8 = np.zeros((128, 8), np.float32)
    for p in range(128):
        mask8[p, p // GSIZE] = 1.0
    sel8 = mask8 / GSIZE
    repl8 = np.ascontiguousarray(mask8.T)
    wkf = np.asarray(wk, np.float32)
    wqf = np.asarray(wq, np.float32)
    gmat = ((wqf @ wkf.T) * SCALE).astype(BFh)
    bqwk = ((np.asarray(bq, np.float32) @ wkf.T) * SCALE).astype(np.float32)
    E4h = ml_dtypes.float8_e4m3
    wvf = np.asarray(wv, np.float32)
    wpf = np.asarray(wp, np.float32)
    # [cp, 128, i, C]: chunk-pair DoubleRow layouts for the V / out projections
    wv8 = np.ascontiguousarray(
        wvf.reshape(2, 2, 128, C).transpose(0, 2, 1, 3)).astype(E4h)
    wp8 = np.ascontiguousarray(
        wpf.reshape(2, 2, 128, C).transpose(0, 2, 1, 3)).astype(E4h)
    bpp = (np.asarray(bv, np.float32) @ wpf + np.asarray(bp, np.float32)).astype(np.float32)
    common = {
        "gmat": gmat,
        "wv8": wv8, "wp8": wp8, "bpp": bpp,
        "gamma": np.asarray(gamma, np.float32),
        "beta": np.asarray(beta, np.float32),
        "sel8": sel8, "repl8": repl8, "bqwk": bqwk,
    }
    xf = x.reshape(B, T, C)
    in_maps = []
    for core in range(NCORES):
        b, qoff = core // 4, (core % 4) * QS
        # rotate so this core's query strip is rows 0..QS-1 (attention and
        # group stats are permutation-invariant over tokens), then go
        # channel-major for direct DMA into the resident XT tiles
        xr = np.roll(xf[b], -qoff, axis=0)
        in_maps.append({
            **common,
            "xt": np.ascontiguousarray(xr.T.astype(BFh)),
            "xres": np.ascontiguousarray(xf[b, qoff:qoff + QS]),
        })
    return in_maps, fold_q


def kernel(x, gamma, beta, wq, bq, wk, bk, wv, bv, wp, bp):
    in_maps, fold_q = prepare_in_maps(x, gamma, beta, wq, bq, wk, bk, wv, bv, wp, bp)
    if fold_q not in _NC_CACHE:
        _NC_CACHE[fold_q] = _build(fold_q)
    nc = _NC_CACHE[fold_q]
    res = run_bass_kernel_spmd(nc, in_maps, list(range(NCORES)))
    out = np.empty((B, T, C), np.float32)
    for core in range(NCORES):
        b, qoff = core // 4, (core % 4) * QS
        out[b, qoff:qoff + QS] = res.results[core]["out"]
    return out.reshape(B, H, W, C)


# revision 34
# speedup vs baseline: 1.2590x; 1.0156x over previous
"""AttentionBlock (GroupNorm + single-head full attention + residual) on 8 trn2 cores.

Sharding: core i -> batch i//4, query strip (i%4)*1024 .. +1024. Each core
computes its batch's full K/V (duplicated across the 4 cores sharing the
batch) so no inter-core communication is needed. The host rotates each
core's copy of x so its query strip sits at token rows 0..1023 (group-norm
statistics and attention key-sums are permutation-invariant over tokens),
which lets one SPMD program serve all cores.

Changes vs the bf16 baseline (292-352us) -> 173us at full clock:
  - x arrives channel-major (host-side transpose): no PE transposes and no
    ones/Square stats matmuls. Group-norm stats: DVE bn_stats/bn_aggr for
    chunks 0-2; chunk 3 entirely on ScalarE via two Square+accum_out
    passes (sum(x) recovered from sum((x+1)^2)-sum(x^2)-n). Group combine
    and per-channel redistribution are tiny 8/128-partition matmuls
    (1/16 folded into the host sel8 matrix); rstd is plain sqrt+reciprocal
    (~4e-3, inside the fp8 error budget).
  - The Q projection is folded into the K side on the host (when bq == 0):
    Z = h @ (wk wq^T * C^-0.5), so scores S^T = Z^T . h_q use raw
    normalized h on the query side. One less projection pass.
  - The attention core runs in fp8 e4m3 with DoubleRow double-pumped
    matmuls (2 contraction chunks per instruction, issue rate measured at
    the 216ns N=512 streaming floor = 2x bf16 math): scores, exp row-sums
    and P^T.V. Z^T / V / h_q are quantized to e4m3 at PSUM evacuation.
    exp(s - 5) keeps P in e4m3 range (logits measured in [-7.5, 7.2];
    e4m3 covers [2e-3, 240]). Softmax normalization is deferred to the
    f32r projection output, so fp8 rowsum noise largely cancels.
  - P3 is software-pipelined per 256-key pair: scores/exp for pair p while
    the rowsum + P^T.V DoubleRow matmuls consume pair p-1, so the PE never
    paces on the ScalarE exp drain. PSUM: 4 ot banks + 1 rowsum + 3 score.
  - Only ONE normalized activation tensor exists: h8, e4m3 in DoubleRow
    pair layout. It feeds the V projection (full fp8 DoubleRow), the
    score query side, and the Z projection (bf16 G weights x e4m3 h at
    bf16 rate: quantizing G too would fail the gate at 3e-2, numpy-model).
    The out-projection also runs fp8 DoubleRow on e4m3-evacuated ot.
    bv@wp+bp is folded on the host. Evacuations are spread DVE/ScalarE;
    P2 ScalarE is all-Identity and P3 ScalarE is pure-Exp (act-table
    reloads cost ~1.3us each).
End-to-end absmax-relative error vs the fp32 reference: 5.9e-3 on HW
(tolerance 2e-2). HAM keep-alive matmuls hold the PE clock at 2.4GHz.
Measured 173.4us at full clock; the device alternates into a ~2.0GHz
P0 power regime under sustained load where the same NEFF reads ~204us.
"""

import numpy as np
from contextlib import ExitStack

import concourse.bass as bass
import concourse.bacc as bacc
import concourse.tile as tile
from concourse import mybir
from concourse.bass_utils import run_bass_kernel_spmd

B, H, W, C = 2, 64, 64, 512
T = H * W                 # 4096 tokens per batch
NCORES = 8
QS = 1024                 # queries per core
GROUPS, GSIZE = 32, 16    # 8 groups per 128-channel chunk
EPS = 1e-5
SCALE = float(C) ** -0.5
SHIFT = 5.0               # softmax logit shift so exp() fits e4m3
F32 = mybir.dt.float32
F32R = mybir.dt.float32r
BF16 = mybir.dt.bfloat16
E4 = mybir.dt.float8e4
DR = mybir.MatmulPerfMode.DoubleRow
NCH = C // 128            # 4 channel chunks
NW = T // 512             # 8 token windows per batch
NBLK = QS // 512          # 2 attention q-blocks of 512 queries
NSUB = 4                  # 128-query subtiles per block
NKP = T // 256            # 16 key-tile pairs per q-block


def _build(fold_q: bool):
    nc = bacc.Bacc(None, target_bir_lowering=False)

    xt_h = nc.declare_dram_parameter("xt", [C, T], BF16, isOutput=False)
    xres_h = nc.declare_dram_parameter("xres", [QS, C], F32, isOutput=False)
    g_h = nc.declare_dram_parameter("gmat", [C, C], BF16, isOutput=False)
    wv8_h = nc.declare_dram_parameter("wv8", [2, 128, 2, C], E4, isOutput=False)
    wp8_h = nc.declare_dram_parameter("wp8", [2, 128, 2, C], E4, isOutput=False)
    bpp_h = nc.declare_dram_parameter("bpp", [C], F32, isOutput=False)
    gamma_h = nc.declare_dram_parameter("gamma", [C], F32, isOutput=False)
    beta_h = nc.declare_dram_parameter("beta", [C], F32, isOutput=False)
    sel8_h = nc.declare_dram_parameter("sel8", [128, 8], F32, isOutput=False)
    repl8_h = nc.declare_dram_parameter("repl8", [8, 128], F32, isOutput=False)
    bqwk_h = nc.declare_dram_parameter("bqwk", [C], F32, isOutput=False)
    out_h = nc.declare_dram_parameter("out", [QS, C], F32, isOutput=True)

    with tile.TileContext(nc) as tc, ExitStack() as ctx:
        persist = ctx.enter_context(tc.tile_pool(name="persist", bufs=1))
        small = ctx.enter_context(tc.tile_pool(name="small", bufs=1))

        bigpool = ctx.enter_context(tc.tile_pool(name="bigpool", bufs=1))
        xt_t = [bigpool.tile([128, T], BF16, tag=f"xt{j}", name=f"xt{j}") for j in range(NCH)]
        # fp8 operand tiles in DoubleRow pair layout [128, 2, ...]
        y8 = [bigpool.tile([128, 2, QS], E4, tag=f"y8{c}", name=f"y8{c}") for c in range(2)]
        h8 = [bigpool.tile([128, 2, T], E4, tag=f"h8{c}", name=f"h8{c}") for c in range(2)]
        v_big = bigpool.tile([128, T // 128, C], E4, tag="vbig", name="vbig")

        ctx2 = ExitStack()
        wpool = ctx2.enter_context(tc.tile_pool(name="wpool", bufs=1))
        g_t = [wpool.tile([128, C], BF16, tag=f"g{j}", name=f"g{j}") for j in range(NCH)]
        wv8_t = [wpool.tile([128, 2, C], E4, tag=f"wv8{j}", name=f"wv8{j}") for j in range(2)]
        wp8_t = [persist.tile([128, 2, C], E4, tag=f"wp8{j}", name=f"wp8{j}") for j in range(2)]
        for j in range(NCH):
            sl = slice(j * 128, (j + 1) * 128)
            nc.scalar.dma_start(out=g_t[j], in_=g_h[sl, :])
        for cp in range(2):
            nc.scalar.dma_start(out=wv8_t[cp], in_=wv8_h[cp, :, :, :])
            nc.scalar.dma_start(out=wp8_t[cp], in_=wp8_h[cp, :, :, :])

        # per-channel vectors as [128, NCH] (column j = channel chunk j)
        def vec_tile(h, name):
            t = small.tile([128, NCH], F32, tag=name)
            nc.scalar.dma_start(out=t, in_=h.rearrange("(a p) -> p a", p=128))
            return t

        gamma_sb = vec_tile(gamma_h, "gamma")
        beta_sb = vec_tile(beta_h, "beta")
        bpp_row = small.tile([1, C], F32, tag="bpprow", name="bpprow")
        nc.scalar.dma_start(out=bpp_row, in_=bpp_h.rearrange("(a c) -> a c", a=1))
        sel8 = small.tile([128, 8], F32, tag="sel8", name="sel8")
        nc.sync.dma_start(out=sel8, in_=sel8_h[:, :])
        repl8 = small.tile([8, 128], F32, tag="repl8", name="repl8")
        nc.sync.dma_start(out=repl8, in_=repl8_h[:, :])
        bqwk_sb = vec_tile(bqwk_h, "bqwk")

        ones1 = small.tile([1, 1], F32, tag="ones1", name="ones1")
        nc.vector.memset(ones1, 1.0)
        nshift = small.tile([128, 1], F32, tag="nshift", name="nshift")
        nc.vector.memset(nshift, -SHIFT)
        onesd = small.tile([128, 2, 16], E4, tag="onesd", name="onesd")
        nc.vector.memset(onesd, 1.0)

        rinv_t = [small.tile([128, 1], F32, tag=f"rinv{s}", name=f"rinv{s}") for s in range(NSUB * NBLK)]
        scale_t = [small.tile([128, 1], F32, tag=f"gnsc{j}", name=f"gnsc{j}") for j in range(NCH)]
        bias_t = [small.tile([128, 1], F32, tag=f"gnbi{j}", name=f"gnbi{j}") for j in range(NCH)]

        # PE warm-up / keep-alive dummy matmuls (HAM unthrottle 1.2->2.4GHz)
        warm_sb = small.tile([128, 512], BF16, tag="warm_sb", name="warm_sb")
        nc.vector.memset(warm_sb, 0.0)

        ctxkeep = ExitStack()
        p1ps_keep = ctxkeep.enter_context(tc.tile_pool(name="keepps", bufs=1, space="PSUM"))

        def keepalive(n, lhs=None):
            for _ in range(n):
                kps = p1ps_keep.tile([128, 512], F32, tag="keep", name="keep", bufs=1)
                if lhs is None:
                    nc.tensor.matmul(kps, warm_sb[:, 0:128], warm_sb,
                                     start=True, stop=True)
                else:
                    nc.tensor.matmul(kps[0:1, :], lhs, warm_sb,
                                     start=True, stop=True)

        # ================= P1: stream XT, bn_stats group statistics =========
        # Per-chunk pipeline (a chunk's group scale/bias only depends on its
        # own 128 channels): half-chunk DMAs -> 3D bn_stats -> bn_aggr ->
        # tiny 8/128-partition matmuls for the 16-channel group combine and
        # per-channel redistribution.
        with tc.tile_pool(name="p1ps", bufs=1, space="PSUM") as p1ps, \
             tc.tile_pool(name="p1sb", bufs=1) as p1sb:
            keepalive(18)
            HT = T // 2
            # DMA halves interleaved so the ScalarE/reduce chunk (3) lands
            # early while the DVE bn_stats chunks stream in order
            for hf in range(2):
                for j in (3, 0, 1, 2):
                    sl = slice(hf * HT, (hf + 1) * HT)
                    nc.sync.dma_start(out=xt_t[j][:, sl],
                                      in_=xt_h[j * 128:(j + 1) * 128, sl])

            Sjs = []
            # chunk 3 entirely on ScalarE: two Square+accum passes per slice
            # (bias 0 and bias 1; one act-table segment). Algebra recovers the
            # plain sums: sum(x) = (sum((x+1)^2) - sum(x^2) - 512) / 2.
            onescol = small.tile([128, 1], F32, tag="onescol", name="onescol")
            nc.vector.memset(onescol, 1.0)
            ssq8 = p1sb.tile([128, NW], F32, tag="ssq8", name="ssq8")
            sqb8 = p1sb.tile([128, NW], F32, tag="sqb8", name="sqb8")
            for s in range(NW):
                scr = p1sb.tile([128, 512], BF16, tag="sqscr", name="sqscr", bufs=2)
                nc.scalar.activation(scr, xt_t[3][:, s * 512:(s + 1) * 512],
                                     mybir.ActivationFunctionType.Square,
                                     accum_out=ssq8[:, s:s + 1])
                scr2 = p1sb.tile([128, 512], BF16, tag="sqscr", name="sqscr", bufs=2)
                nc.scalar.activation(scr2, xt_t[3][:, s * 512:(s + 1) * 512],
                                     mybir.ActivationFunctionType.Square,
                                     bias=onescol,
                                     accum_out=sqb8[:, s:s + 1])
            # chunks 0-2: DVE bn_stats / bn_aggr
            for j in range(3):
                bns = p1sb.tile([128, NW, 6], F32, tag=f"bns{j}", name=f"bns{j}")
                for s in range(NW):
                    nc.vector.bn_stats(bns[:, s, :],
                                       xt_t[j][:, s * 512:(s + 1) * 512])
                mvj = p1sb.tile([128, 2], F32, tag=f"mv{j}", name=f"mv{j}")
                nc.vector.bn_aggr(mvj, bns)
                # Sj: col 0 = mean_c, col 1 = E[x^2]_c
                Sj = p1sb.tile([128, 2], F32, tag=f"S{j}", name=f"S{j}")
                nc.vector.tensor_copy(Sj[:, 0:1], mvj[:, 0:1])
                nc.vector.scalar_tensor_tensor(
                    out=Sj[:, 1:2], in0=mvj[:, 0:1], scalar=mvj[:, 0:1],
                    in1=mvj[:, 1:2], op0=mybir.AluOpType.mult,
                    op1=mybir.AluOpType.add)
                Sjs.append(Sj)
                if j in (0, 1, 2):
                    wj = p1sb.tile([128, 1], BF16, tag=f"warm{j}", name=f"warm{j}")
                    nc.gpsimd.tensor_copy(wj, mvj[:, 0:1])
                    keepalive(3, lhs=wj)
            S3 = p1sb.tile([128, 2], F32, tag="S3", name="S3")
            dsq = p1sb.tile([128, NW], F32, tag="dsq", name="dsq")
            nc.vector.tensor_tensor(out=dsq, in0=sqb8, in1=ssq8,
                                    op=mybir.AluOpType.subtract)
            nc.vector.tensor_reduce(out=S3[:, 0:1], in_=dsq,
                                    axis=mybir.AxisListType.X, op=mybir.AluOpType.add)
            # sum(dsq) = 2*sum(x) + T  ->  mean = (sum(dsq) - T) / (2T)
            nc.vector.tensor_scalar(out=S3[:, 0:1], in0=S3[:, 0:1],
                                    scalar1=-float(T), scalar2=0.5 / T,
                                    op0=mybir.AluOpType.add,
                                    op1=mybir.AluOpType.mult)
            nc.vector.tensor_reduce(out=S3[:, 1:2], in_=ssq8,
                                    axis=mybir.AxisListType.X, op=mybir.AluOpType.add)
            nc.vector.tensor_scalar_mul(S3[:, 1:2], S3[:, 1:2], 1.0 / T)
            Sjs.append(S3)
            w3 = p1sb.tile([128, 1], BF16, tag="warm3", name="warm3")
            nc.gpsimd.tensor_copy(w3, S3[:, 0:1])
            keepalive(3, lhs=w3)
            # pre-load the Exp activation table so the first P3 exp doesn't
            # pay the ~1.3us table switch inside the score pipeline
            expwarm = p1sb.tile([128, 1], F32, tag="expwarm", name="expwarm")
            nc.scalar.activation(expwarm, nshift,
                                 mybir.ActivationFunctionType.Exp)

            # per chunk: group-combine matmul (1/16 folded into sel8),
            # plain sqrt+reciprocal rstd (~4e-3, inside the fp8 budget),
            # redistribution matmul, then per-channel scale/bias
            for j, Sj in zip((0, 1, 2, 3), Sjs):
                g8_ps = p1ps.tile([8, 2], F32, tag="g8", name="g8", bufs=2)
                nc.tensor.matmul(g8_ps, sel8, Sj, start=True, stop=True)
                vals = p1sb.tile([8, 2], F32, tag=f"vals{j}", name=f"vals{j}")
                nc.vector.tensor_copy(vals, g8_ps)
                msq8 = p1sb.tile([8, 1], F32, tag="msq8", name="msq8")
                nc.vector.tensor_tensor(out=msq8, in0=vals[:, 0:1], in1=vals[:, 0:1],
                                        op=mybir.AluOpType.mult)
                ve = p1sb.tile([8, 1], F32, tag="ve", name="ve")
                nc.vector.tensor_tensor(out=ve, in0=vals[:, 1:2], in1=msq8,
                                        op=mybir.AluOpType.subtract)
                nc.vector.tensor_scalar_add(ve, ve, EPS)
                sd = p1sb.tile([8, 1], F32, tag="sd", name="sd")
                nc.scalar.activation(sd, ve, mybir.ActivationFunctionType.Sqrt)
                nc.vector.reciprocal(vals[:, 1:2], sd)
                b128_ps = p1ps.tile([128, 2], F32, tag="b128", name="b128", bufs=2)
                nc.tensor.matmul(b128_ps, repl8, vals, start=True, stop=True)
                bc = p1sb.tile([128, 2], F32, tag=f"bc{j}", name=f"bc{j}")
                nc.vector.tensor_copy(bc, b128_ps)
                nc.vector.tensor_tensor(out=scale_t[j], in0=bc[:, 1:2],
                                        in1=gamma_sb[:, j:j + 1],
                                        op=mybir.AluOpType.mult)
                mt = p1sb.tile([128, 1], F32, tag="mt", name="mt")
                nc.vector.tensor_tensor(out=mt, in0=bc[:, 0:1], in1=scale_t[j],
                                        op=mybir.AluOpType.mult)
                nc.vector.tensor_tensor(out=bias_t[j], in0=beta_sb[:, j:j + 1],
                                        in1=mt, op=mybir.AluOpType.subtract)

        keepalive(4)
        ctxkeep.close()

        # ================= P2: normalize window-pairs -> Z^T, V (+ Q^T) =====
        # 1024-token moving operands (bf16 moving max) into 2-bank PSUM
        # tiles: half the matmul/LDWEIGHTS/evac instruction count.
        with tc.tile_pool(name="p2ps", bufs=1, space="PSUM") as p2ps, \
             tc.tile_pool(name="p2sb", bufs=2) as p2sb:
            for wp in range(NW // 2):
                t0 = wp * 1024
                # normalize straight into the e4m3 pair-layout h8 (only
                # e4m3 h exists; Y runs bf16-weights x e4m3-h at bf16 rate)
                for j in range(2):
                    nc.vector.tensor_scalar(
                        out=h8[j // 2][:, j % 2, t0:t0 + 1024],
                        in0=xt_t[j][:, t0:t0 + 1024],
                        scalar1=scale_t[j], scalar2=bias_t[j],
                        op0=mybir.AluOpType.mult, op1=mybir.AluOpType.add)
                nc.scalar.activation(
                    h8[1][:, 0, t0:t0 + 1024], xt_t[2][:, t0:t0 + 1024],
                    mybir.ActivationFunctionType.Identity,
                    bias=bias_t[2], scale=scale_t[2])
                nc.gpsimd.tensor_scalar(
                    out=h8[1][:, 1, t0:t0 + 1024], in0=xt_t[3][:, t0:t0 + 1024],
                    scalar1=scale_t[3], scalar2=bias_t[3],
                    op0=mybir.AluOpType.mult, op1=mybir.AluOpType.add)
                if wp == 0:
                    # Y = h_q (wq wk^T) * scale + bq wk^T : only the 1024
                    # query rows ever need projecting (key side stays raw h8)
                    for yc in range(NCH):
                        ps2 = p2ps.tile([128, 2, 512], F32, tag="zp", name="zp", bufs=2)
                        for hh in range(2):
                            hs = t0 + hh * 512
                            for ci in range(NCH):
                                nc.tensor.matmul(
                                    ps2[:, hh, :], g_t[ci][:, yc * 128:(yc + 1) * 128],
                                    h8[ci // 2][:, ci % 2, hs:hs + 512],
                                    start=(ci == 0), stop=(ci == NCH - 1))
                        nc.vector.tensor_scalar_add(
                            y8[yc // 2][:, yc % 2, :], ps2, bqwk_sb[:, yc:yc + 1])
                for m in range(4):
                    ps2 = p2ps.tile([128, 2, 512], F32, tag="vp", name="vp", bufs=2)
                    for hh in range(2):
                        tb = wp * 8 + 2 * m + hh
                        for cp in range(2):
                            nc.tensor.matmul(
                                ps2[:, hh, :],
                                h8[cp][:, :, tb * 128:(tb + 1) * 128],
                                wv8_t[cp], start=(cp == 0), stop=(cp == 1),
                                perf_mode=DR)
                    if m < 2:
                        nc.vector.tensor_copy(
                            v_big[:, wp * 8 + 2 * m:wp * 8 + 2 * m + 2, :], ps2)
                    else:
                        nc.scalar.activation(
                            v_big[:, wp * 8 + 2 * m:wp * 8 + 2 * m + 2, :], ps2,
                            mybir.ActivationFunctionType.Identity)
        ctx2.close()

        # ================= P3: fp8 DoubleRow attention =======================
        otspool = ctx.enter_context(tc.tile_pool(name="otspool", bufs=1))
        ots8_t = [[otspool.tile([128, 2, 512], E4, tag=f"ots{b}_{cp}", name=f"ots{b}_{cp}")
                   for cp in range(2)] for b in range(NBLK)]
        with tc.tile_pool(name="p3ps", bufs=1, space="PSUM") as p3ps, \
             tc.tile_pool(name="p3ot", bufs=1, space="PSUM") as p3ot, \
             tc.tile_pool(name="p3sb", bufs=1) as p3sb, \
             tc.tile_pool(name="p3ac", bufs=4) as p3ac:
            # bias vector bp' = bv @ wp + bp (host-folded), broadcast
            bppb = p3sb.tile([128, C], F32, tag="bppb", name="bppb")
            nc.gpsimd.partition_broadcast(bppb, bpp_row[0:1, :])

            for blk in range(NBLK):
                q0 = blk * 512
                ot_ps = p3ot.tile([128, NCH, 512], F32, tag="ot", name="ot", bufs=1)
                rs_ps = p3ot.tile([1, 512], F32, tag="rsum", name="rsum", bufs=1)
                pts = [None] * NKP

                # software-pipelined: scores/exp for key-pair p while the
                # rowsum + P^T.V matmuls consume pair p-1 (PE stays busy
                # instead of pacing on the ScalarE exp drain)
                def scores_pair(p):
                    for hh in range(2):
                        w2 = 2 * p + hh
                        st_ps = p3ps.tile([128, 512], F32, tag="sc", name="st_ps", bufs=3)
                        for c2 in range(2):
                            nc.tensor.matmul(
                                st_ps, h8[c2][:, :, w2 * 128:(w2 + 1) * 128],
                                y8[c2][:, :, q0:q0 + 512],
                                start=(c2 == 0), stop=(c2 == 1), perf_mode=DR)
                        if hh == 0:
                            pts[p] = p3sb.tile([128, 2, 512], E4, tag="pt",
                                               name="pt", bufs=8)
                        nc.scalar.activation(pts[p][:, hh, :], st_ps,
                                             mybir.ActivationFunctionType.Exp,
                                             bias=nshift)

                def consume_pair(p):
                    nc.tensor.matmul(rs_ps, onesd[:, :, 0:1], pts[p],
                                     start=(p == 0), stop=(p == NKP - 1),
                                     perf_mode=DR)
                    for cv in range(NCH):
                        nc.tensor.matmul(
                            ot_ps[:, cv, :],
                            v_big[:, 2 * p:2 * p + 2, cv * 128:(cv + 1) * 128],
                            pts[p], start=(p == 0), stop=(p == NKP - 1),
                            perf_mode=DR)

                for p in range(NKP + 1):
                    if p < NKP:
                        scores_pair(p)
                    if p >= 1:
                        consume_pair(p - 1)

                rs_row = p3sb.tile([1, 512], F32, tag="rs_row", name="rs_row", bufs=2)
                nc.vector.tensor_copy(rs_row, rs_ps)
                for sub in range(NSUB):
                    rt_ps = p3ps.tile([128, 1], F32, tag="sc", name="rt", bufs=3)
                    nc.tensor.transpose(
                        rt_ps, rs_row[0:1, sub * 128:(sub + 1) * 128], ones1)
                    rr = p3ac.tile([128, 1], F32, tag="rr", name="rr")
                    nc.vector.tensor_copy(rr, rt_ps)
                    nc.vector.reciprocal(rinv_t[blk * NSUB + sub], rr)
                # DVE-only evacuation: keeps P3's ScalarE queue pure-Exp
                # (no act-table reloads between blocks)
                nc.vector.tensor_copy(ots8_t[blk][0][:, :, :], ot_ps[:, 0:2, :])
                nc.vector.tensor_copy(ots8_t[blk][1][:, :, :], ot_ps[:, 2:4, :])

                for sub in range(NSUB):
                    ti = blk * NSUB + sub
                    ps_p = p3ps.tile([128, C], F32, tag="sc", name="ps_p", bufs=3)
                    for cp in range(2):
                        nc.tensor.matmul(
                            ps_p, ots8_t[blk][cp][:, :, sub * 128:(sub + 1) * 128],
                            wp8_t[cp], start=(cp == 0), stop=(cp == 1),
                            perf_mode=DR)
                    xres = p3sb.tile([128, C], F32, tag="xres", name="xres", bufs=3)
                    nc.sync.dma_start(out=xres, in_=xres_h[ti * 128:(ti + 1) * 128, :])
                    tmp = p3sb.tile([128, C], F32, tag="tmp", name="tmp", bufs=3)
                    nc.vector.scalar_tensor_tensor(
                        out=tmp, in0=ps_p, scalar=rinv_t[ti], in1=xres,
                        op0=mybir.AluOpType.mult, op1=mybir.AluOpType.add)
                    fin = p3sb.tile([128, C], F32, tag="fin", name="fin", bufs=3)
                    nc.vector.tensor_tensor(out=fin, in0=tmp, in1=bppb,
                                            op=mybir.AluOpType.add)
                    nc.sync.dma_start(out=out_h[ti * 128:(ti + 1) * 128, :], in_=fin)

    nc.compile()
    return nc


_NC_CACHE = {}


def prepare_in_maps(x, gamma, beta, wq, bq, wk, bk, wv, bv, wp, bp):
    import ml_dtypes
    BFh = ml_dtypes.bfloat16
    x = np.ascontiguousarray(np.asarray(x, dtype=np.float32))
    fold_q = False
    # sel8 folds the 1/16 group average; repl8 is the binary redistribution
    mask8 = np.zeros((128, 8), np.float32)
    for p in range(128):
        mask8[p, p // GSIZE] = 1.0
    sel8 = mask8 / GSIZE
    repl8 = np.ascontiguousarray(mask8.T)
    wkf = np.asarray(wk, np.float32)
    wqf = np.asarray(wq, np.float32)
    gmat = ((wqf @ wkf.T) * SCALE).astype(BFh)
    bqwk = ((np.asarray(bq, np.float32) @ wkf.T) * SCALE).astype(np.float32)
    E4h = ml_dtypes.float8_e4m3
    wvf = np.asarray(wv, np.float32)
    wpf = np.asarray(wp, np.float32)
    # [cp, 128, i, C]: chunk-pair DoubleRow layouts for the V / out projections
    wv8 = np.ascontiguousarray(
        wvf.reshape(2, 2, 128, C).transpose(0, 2, 1, 3)).astype(E4h)
    wp8 = np.ascontiguousarray(
        wpf.reshape(2, 2, 128, C).transpose(0, 2, 1, 3)).astype(E4h)
    bpp = (np.asarray(bv, np.float32) @ wpf + np.asarray(bp, np.float32)).astype(np.float32)
    common = {
        "gmat": gmat,
        "wv8": wv8, "wp8": wp8, "bpp": bpp,
        "gamma": np.asarray(gamma, np.float32),
        "beta": np.asarray(beta, np.float32),
        "sel8": sel8, "repl8": repl8, "bqwk": bqwk,
    }
    xf = x.reshape(B, T, C)
    in_maps = []
    for core in range(NCORES):
        b, qoff = core // 4, (core % 4) * QS
        # rotate so this core's query strip is rows 0..QS-1 (attention and
        # group stats are permutation-invariant over tokens), then go
        # channel-major for direct DMA into the resident XT tiles
        xr = np.roll(xf[b], -qoff, axis=0)
        in_maps.append({
            **common,
            "xt": np.ascontiguousarray(xr.T.astype(BFh)),
            "xres": np.ascontiguousarray(xf[b, qoff:qoff + QS]),
        })
    return in_maps, fold_q


def kernel(x, gamma, beta, wq, bq, wk, bk, wv, bv, wp, bp):
    in_maps, fold_q = prepare_in_maps(x, gamma, beta, wq, bq, wk, bk, wv, bv, wp, bp)
    if fold_q not in _NC_CACHE:
        _NC_CACHE[fold_q] = _build(fold_q)
    nc = _NC_CACHE[fold_q]
    res = run_bass_kernel_spmd(nc, in_maps, list(range(NCORES)))
    out = np.empty((B, T, C), np.float32)
    for core in range(NCORES):
        b, qoff = core // 4, (core % 4) * QS
        out[b, qoff:qoff + QS] = res.results[core]["out"]
    return out.reshape(B, H, W, C)
